# revision 30
# baseline (speedup 1.0000x reference)
"""Trainium2 Bass kernel for a dense transformer block (pre-LN, MHA + GELU MLP).

Problem shapes (hardcoded): x [2, 2048, 768] f32, mask [2, 2048] int32,
12 heads x 64 dims, hidden 3072.

Sharding: 8 cores = (batch b in {0,1}) x (token shard s in {0..3}).
Each core handles its 512-query shard of Q / attention rows / MLP / output.

Key compaction: the key-padding mask kills ~half the keys, so the host
gathers only the unmasked keys' x columns into xk [D, NK] (NK = count
rounded up to 256, zero-padded). Pad keys are neutralized by zeroing their
V rows and ones-column entries.

LN1 is never applied to activations ("raw-G" trick): Q/K/V matmuls run on
raw fp8 x directly (so the PE starts as soon as DMA lands, with no
LN-apply serialization), and the normalization is folded into the
epilogues via per-output-channel weight column sums:
    q[o,t] = r[t] * (G[o,t] - m[t]*colsum_w[o])        (valid: ln1_b == 0)
The query-side rstd folds into the Q epilogue; the key-side rstd folds
into the exp's per-partition ACT scale (keys sit on PSUM partitions for
scores), and the V-side mean/rstd fold into the V epilogue using
transposed per-key mean/rstd columns (tiny PE transpose matmuls).

On-chip layout is feature-major: activations are [features, tokens],
every matmul contracts over the partition dim. Per-token LN stats come
from ones-vector DoubleRow matmuls on raw fp8 x; the fp8 squares are
split across DVE/ACT/GpSimd so no one engine serializes the startup.
Softmax denominators come free from a ones column appended to V (M=65
matmul). Max-subtraction is skipped: |scores| <= ~4 by construction.
LN2 mean is folded into the proj PSUM as a K=1 rank-1 accumulate (and
un-folded in the fc2 PSUM), so the residual stream lives centered (d2)
and the LN2 apply is one multiply.

fp8 DoubleRow everywhere except fc2 (kept bf16: quantizing the 3072-wide
contraction would eat too much of the 2e-2 error budget). DMAs are one
issue per tensor (descriptor issue costs ~600ns on the Sync engine) in
consumer-priority order. Perf note for this box: the PE runs at 1.2 GHz
through dependency-mixed regions (only long uniform MM streams reach
2.4 GHz), so attention-path epilogues belong on DVE, not on the PE.
"""

import numpy as np
import ml_dtypes

import concourse.bass as bass
import concourse.tile as tile
import concourse.mybir as mybir
from concourse import bacc
from concourse.bass import ts
from concourse.bass_utils import run_bass_kernel_spmd
from concourse.alu_op_type import AluOpType

BF16 = mybir.dt.bfloat16
F32 = mybir.dt.float32
FP8 = mybir.dt.float8e4
DR = mybir.MatmulPerfMode.DoubleRow
WS = 32.0   # fp8 weight scale (dodges e4m3 subnormals)

B = 2
N = 2048
D = 768
H = 12
HD = 64
HID = 3072
EPS = 1e-5
SCALE = HD ** -0.5
NQ = 512          # queries per core
NSH = N // NQ     # token shards per batch
NC = B * NSH      # 8 cores
C6 = D // 128     # feature chunks
HO24 = HID // 128

AF = mybir.ActivationFunctionType
OP = AluOpType

_cached = {}
_rid = [0]


def _rid_next():
    _rid[0] += 1
    return _rid[0]


def _build_nc(sbp, NK):
    nc = bacc.Bacc("TRN2", target_bir_lowering=False, debug=False,
                   enable_asserts=False, num_devices=NC)

    xq8 = nc.dram_tensor("xq8", [D, NQ], FP8, kind="ExternalInput").ap()
    xk8 = nc.dram_tensor("xk8", [D, NK], FP8, kind="ExternalInput").ap()
    xb = nc.dram_tensor("xb", [D, NQ], BF16, kind="ExternalInput").ap()
    wqkv = nc.dram_tensor("wqkv", [D, 3 * D], FP8, kind="ExternalInput").ap()
    wproj = nc.dram_tensor("wproj", [D, D], FP8, kind="ExternalInput").ap()
    wfc1 = nc.dram_tensor("wfc1", [D, HID], FP8, kind="ExternalInput").ap()
    wfc2 = nc.dram_tensor("wfc2", [HID, D], BF16, kind="ExternalInput").ap()
    uproj = nc.dram_tensor("uproj", [128, 6], BF16, kind="ExternalInput").ap()
    # negated WS-scaled weight column sums as rows: csq | csk | csv
    csrows = nc.dram_tensor("csrows", [1, 3 * D], BF16,
                            kind="ExternalInput").ap()
    KC = NK // 128
    # packed f32 constants: m01 | csq | csk | bfc1 | bfc2
    constf = nc.dram_tensor("constf", [128, KC + 42], F32,
                            kind="ExternalInput").ap()
    out_d = nc.dram_tensor("out", [D, NQ], F32, kind="ExternalOutput").ap()
    import os
    dbg = {}
    if os.environ.get("KDBG"):
        dbg["y"] = nc.dram_tensor("dbg_y", [D, NQ], F32, kind="ExternalOutput").ap()
        dbg["x2"] = nc.dram_tensor("dbg_x2", [D, NQ], F32, kind="ExternalOutput").ap()
        dbg["q"] = nc.dram_tensor("dbg_q", [D, NQ], F32, kind="ExternalOutput").ap()
        dbg["k"] = nc.dram_tensor("dbg_k", [D, NK], F32, kind="ExternalOutput").ap()
        dbg["xn2"] = nc.dram_tensor("dbg_xn2", [D, NQ], F32, kind="ExternalOutput").ap()

    with tile.TileContext(nc) as tc:
        _body(nc, tc, sbp, NK, xq8, xk8, xb, wqkv, wproj, wfc1, wfc2,
              uproj, csrows, constf, out_d, dbg)
    nc.compile()
    return nc


def _body(nc, tc, sbp, NK, xq8_d, xk8_d, xb_d, wqkv_d, wproj_d, wfc1_d,
          wfc2_d, uproj_d, csrows_d, constf_d, out_d, dbg=None):
    dbg = dbg or {}
    KC = NK // 128
    NPAIR = KC // 2
    KT = []
    off = 0
    while off < NK:
        w = min(512, NK - off)
        KT.append((off, w))
        off += w
    NT = len(KT)

    P1_cm = tc.tile_pool(name="p1", bufs=1); P1 = P1_cm.__enter__()
    P2_cm = tc.tile_pool(name="p2", bufs=2); P2 = P2_cm.__enter__()
    P4_cm = tc.tile_pool(name="p4", bufs=4); P4 = P4_cm.__enter__()
    P6_cm = tc.tile_pool(name="p6", bufs=6); P6 = P6_cm.__enter__()
    ps_mm_cm = tc.tile_pool(name="ps_mm", bufs=4, space="PSUM")
    ps_mm = ps_mm_cm.__enter__()
    ps_sc_cm = tc.tile_pool(name="ps_sc", bufs=2, space="PSUM")
    ps_sc = ps_sc_cm.__enter__()

    # ---- DMAs: one issue per tensor (issue cost ~600ns each on Sync),
    # ordered by when the consumer needs the data ----
    xq8_r = xq8_d.rearrange("(a p) t -> p a t", p=128)
    xk8_r = xk8_d.rearrange("(a p) t -> p a t", p=128)
    wqkv_r = wqkv_d.rearrange("(a p) o -> p a o", p=128)
    x8 = P1.tile([128, C6, NQ], FP8, tag="x8")
    nc.sync.dma_start(out=x8, in_=xq8_r)
    w_sb = P2.tile([128, C6, 3 * D], FP8, tag="wbig", bufs=2)
    nc.sync.dma_start(out=w_sb[:, :, 0:D], in_=wqkv_r[:, :, 0:D])
    xk8 = P1.tile([128, C6, NK], FP8, tag="xk8")
    for (o2, w2) in KT:
        nc.sync.dma_start(out=xk8[:, :, o2:o2 + w2], in_=xk8_r[:, :, o2:o2 + w2])
    # packed f32 constants: m01 | csq | csk | bfc1 | bfc2
    constf = P1.tile([128, KC + 42], F32, tag="constf")
    nc.sync.dma_start(out=constf, in_=constf_d)
    m01 = constf[:, 0:KC]
    csq_s = constf[:, KC:KC + 6]
    csk_s = constf[:, KC + 6:KC + 12]
    bfc1_s = constf[:, KC + 12:KC + 36]
    bfc2_s = constf[:, KC + 36:KC + 42]
    nc.sync.dma_start(out=w_sb[:, :, D:2 * D], in_=wqkv_r[:, :, D:2 * D])
    csr = P1.tile([1, 3 * D], BF16, tag="csr")
    nc.sync.dma_start(out=csr, in_=csrows_d)
    csq_row = csr[:, 0:D]
    csk_row = csr[:, D:2 * D]
    csv_row = csr[:, 2 * D:3 * D]
    nc.sync.dma_start(out=w_sb[:, :, 2 * D:3 * D], in_=wqkv_r[:, :, 2 * D:3 * D])
    uproj_s = P1.tile([128, 6], BF16, tag="uproj")
    nc.sync.dma_start(out=uproj_s, in_=uproj_d)

    # ---- on-chip constants ----
    onesb = P1.tile([128, 128], BF16, tag="onesb")
    nc.vector.memset(onesb, 1.0)
    ones8_t = P1.tile([128, 2, 16], FP8, tag="ones8")
    nc.vector.memset(ones8_t, 1.0)
    ones8 = ones8_t[:, :, 0:1]
    eps1 = P1.tile([1, 1], F32, tag="eps1")
    nc.vector.memset(eps1, EPS)
    sumx0 = P1.tile([1, NQ], BF16, tag="sumx0")

    # ---- later-phase DMAs (behind the attention-critical ones) ----
    xb_sb = P1.tile([128, C6, NQ], BF16, tag="xb")
    nc.sync.dma_start(out=xb_sb, in_=xb_d.rearrange("(a p) t -> p a t", p=128))
    wproj8 = P1.tile([128, C6, D], FP8, tag="wpj")
    nc.sync.dma_start(out=wproj8, in_=wproj_d.rearrange("(a p) o -> p a o", p=128))
    wfc1_sb = P1.tile([128, C6, HID], FP8, tag="wfc1")
    nc.sync.dma_start(out=wfc1_sb, in_=wfc1_d.rearrange("(a p) o -> p a o", p=128))
    wfc2a = P2.tile([128, 12, D], BF16, tag="wbig", bufs=2)
    nc.sync.dma_start(out=wfc2a,
                      in_=wfc2_d.rearrange("(a p) o -> p a o", p=128)[:, 0:12, :])

    # ========== LN stats on raw fp8 x (sum/sumsq via DR matmuls) ==========
    def emit_stats(src, t_off, w, nm, sqeng="dve"):
        i = _rid_next()
        ps_sum = ps_mm.tile([1, w], F32, tag="mm", name=f"pssum{i}")
        ps_sq = ps_mm.tile([1, w], F32, tag="mm", name=f"pssq{i}")
        for ci in range(0, C6, 2):
            nc.tensor.matmul(ps_sum, ones8, src[:, ci:ci + 2, t_off:t_off + w],
                             start=(ci == 0), stop=(ci == C6 - 2), perf_mode=DR)
        sq = P4.tile([128, C6, w], FP8, tag="sq", name=f"sq{i}", bufs=2)
        for ci in range(0, C6, 2):
            if sqeng == "act":
                nc.scalar.activation(sq[:, ci:ci + 2, :],
                                     src[:, ci:ci + 2, t_off:t_off + w],
                                     AF.Square)
            elif sqeng == "gps":
                nc.gpsimd.tensor_tensor(sq[:, ci:ci + 2, :],
                                        src[:, ci:ci + 2, t_off:t_off + w],
                                        src[:, ci:ci + 2, t_off:t_off + w],
                                        op=OP.mult)
            else:
                nc.vector.tensor_tensor(sq[:, ci:ci + 2, :],
                                        src[:, ci:ci + 2, t_off:t_off + w],
                                        src[:, ci:ci + 2, t_off:t_off + w],
                                        op=OP.mult)
        for ci in range(0, C6, 2):
            nc.tensor.matmul(ps_sq, ones8, sq[:, ci:ci + 2, :],
                             start=(ci == 0), stop=(ci == C6 - 2), perf_mode=DR)
        mrow = P6.tile([1, w], BF16, tag="row", bufs=6, name=f"mrow{i}")
        nc.vector.tensor_scalar(out=mrow, in0=ps_sum, scalar1=1.0 / D,
                                scalar2=None, op0=OP.mult)
        m2 = P6.tile([1, w], F32, tag="rowf", bufs=4, name=f"m2_{i}")
        nc.vector.tensor_tensor(m2, mrow, mrow, op=OP.mult)
        vrow = P6.tile([1, w], F32, tag="rowf", bufs=4, name=f"vrow{i}")
        nc.vector.scalar_tensor_tensor(out=vrow, in0=ps_sq, scalar=1.0 / D,
                                       in1=m2, op0=OP.mult, op1=OP.subtract)
        srt = P6.tile([1, w], F32, tag="rowf", bufs=4, name=f"srt{i}")
        nc.scalar.activation(srt, vrow, AF.Sqrt, bias=eps1)
        rf = P6.tile([1, w], F32, tag="rowf", bufs=4, name=f"rf{i}")
        nc.vector.reciprocal_approx_fast(out=rf, in_=srt)
        rrow = P6.tile([1, w], BF16, tag="row", bufs=6, name=f"rrow{i}")
        nc.vector.tensor_copy(out=rrow, in_=rf)
        return ps_sum, mrow, rrow

    # ---- broadcast tiles (PE K=1 matmul + copy) ----
    def emit_bcast(row, w, nm):
        pmb = ps_mm.tile([128, w], F32, tag="mm", name=f"pmb_{nm}")
        nc.tensor.matmul(pmb, onesb[0:1, :], row, start=True, stop=True)
        t = P4.tile([128, w], BF16, tag="bcs", name=f"bc_{nm}", bufs=4)
        nc.vector.tensor_copy(out=t, in_=pmb)
        return t

    # ---- per-key mean/rstd columns via tiny PE transposes ----
    mkc = P1.tile([128, KC], F32, tag="mkc")
    expsc = P1.tile([128, KC], F32, tag="expsc")
    vscale = P1.tile([128, KC], F32, tag="vscale")
    avcol = P1.tile([128, KC], F32, tag="avcol")

    def emit_kcols(t):
        o2, w2 = KT[t]
        nch = w2 // 128
        c0 = o2 // 128
        i = _rid_next()
        pm = ps_mm.tile([128, nch], F32, tag="mm", name=f"pstm{i}")
        pr = ps_mm.tile([128, nch], F32, tag="mm", name=f"pstr{i}")
        for k in range(nch):
            nc.tensor.matmul(pm[:, k:k + 1],
                             kstats[t][1][0:1, k * 128:(k + 1) * 128],
                             onesb[0:1, 0:1], start=True, stop=True)
            nc.tensor.matmul(pr[:, k:k + 1],
                             kstats[t][2][0:1, k * 128:(k + 1) * 128],
                             onesb[0:1, 0:1], start=True, stop=True)
        nc.vector.tensor_copy(out=mkc[:, c0:c0 + nch], in_=pm)
        nc.vector.tensor_scalar(out=expsc[:, c0:c0 + nch], in0=pr,
                                scalar1=1.0 / WS, scalar2=None, op0=OP.mult)
        nc.vector.tensor_tensor(vscale[:, c0:c0 + nch], expsc[:, c0:c0 + nch],
                                m01[:, c0:c0 + nch], op=OP.mult)
        nc.vector.tensor_tensor(avcol[:, c0:c0 + nch], mkc[:, c0:c0 + nch],
                                vscale[:, c0:c0 + nch], op=OP.mult)

    # stats: queries, then key tile 0 (tile 1 deferred until after the
    # first-scores dependencies so exp can start ASAP)
    ps_sum_q, mrow_q, rrow_q = emit_stats(x8, 0, NQ, "q")
    nc.vector.tensor_copy(out=sumx0, in_=ps_sum_q)
    kstats = [None] * NT
    kstats[0] = emit_stats(xk8, KT[0][0], KT[0][1], "k0", sqeng="act")
    mb_k = [None] * NT
    mb_k[0] = emit_bcast(kstats[0][1], KT[0][1], "mk0")
    emit_kcols(0)
    mb_q = emit_bcast(mrow_q, NQ, "mq")
    rbrow = P6.tile([1, NQ], BF16, tag="row", bufs=6, name="rbrow")
    nc.vector.tensor_scalar(out=rbrow, in0=rrow_q, scalar1=1.0 / WS,
                            scalar2=None, op0=OP.mult)
    rb_q = emit_bcast(rbrow, NQ, "rq")

    # ========== Q (raw-G + folded epilogue) ==========
    qT = P2.tile([128, C6, NQ], BF16, tag="qT", bufs=1)
    for co in range(C6):
        ps = ps_mm.tile([128, NQ], F32, tag="mm")
        for ci in range(0, C6, 2):
            nc.tensor.matmul(ps, w_sb[:, ci:ci + 2, ts(co, 128)],
                             x8[:, ci:ci + 2, :],
                             start=(ci == 0), stop=(ci == C6 - 2), perf_mode=DR)
        t1 = P4.tile([128, NQ], BF16, tag="tmp", name=f"qt1_{co}")
        nc.vector.scalar_tensor_tensor(out=t1, in0=mb_q,
                                       scalar=csq_s[:, co:co + 1], in1=ps,
                                       op0=OP.mult, op1=OP.add)
        nc.vector.tensor_tensor(qT[:, co, :], t1, rb_q, op=OP.mult)

    if "q" in dbg:
        for c in range(C6):
            dq_ = P4.tile([128, 512], F32, tag="dbgt", name=f"dbq{c}", bufs=1)
            nc.vector.tensor_copy(out=dq_, in_=qT[:, c, :])
            nc.sync.dma_start(out=dbg["q"][ts(c, 128), :], in_=dq_)

    # ========== attention pipeline ==========
    vsb = P1.tile([128, KC, 16 * ((H * (HD + 1) + 15) // 16)], FP8, tag="vsb")
    m01r = bass.AP(tensor=m01.tensor, offset=m01.offset,
                   ap=[list(m01.ap[0]), list(m01.ap[1]), [0, H], [0, 1]])
    vsb_h = vsb[:, :, 0:H * (HD + 1)].rearrange("p k (h e) -> p k h e", e=HD + 1)
    nc.vector.tensor_copy(out=vsb_h[:, :, :, HD:HD + 1], in_=m01r)

    kch_state = [None]

    def emit_k_chunk_mm(kch_p, p, o2, w2, ci, t):
        if ci == 0:
            kst = ps_mm.tile([128, w2], F32, tag="mm", name=f"kst{_rid_next()}")
            kch_state[0] = kst
        nc.tensor.matmul(kch_state[0], w_sb[:, ci:ci + 2, ts(6 + p, 128)],
                         xk8[:, ci:ci + 2, o2:o2 + w2],
                         start=(ci == 0), stop=(ci == C6 - 2), perf_mode=DR)
        if ci == C6 - 2:
            nc.vector.scalar_tensor_tensor(out=kch_p[:, o2:o2 + w2],
                                           in0=mb_k[t],
                                           scalar=csk_s[:, p:p + 1],
                                           in1=kch_state[0],
                                           op0=OP.mult, op1=OP.add)
            kch_state[0] = None

    def emit_v_chunk(tk):
        i = _rid_next()
        for half in range(2):
            psv = ps_mm.tile([128, 512], F32, tag="mm", name=f"psv{tk}_{half}")
            for ci in range(0, C6, 2):
                nc.tensor.matmul(psv[:, 0:384],
                                 xk8[:, ci:ci + 2, ts(tk, 128)],
                                 w_sb[:, ci:ci + 2, 12 * 128 + half * 384:
                                      12 * 128 + (half + 1) * 384],
                                 start=(ci == 0), stop=(ci == C6 - 2),
                                 perf_mode=DR)
            t1 = P4.tile([128, 384], BF16, tag="vt", name=f"vt{i}_{half}",
                         bufs=4)
            nc.vector.tensor_scalar(out=t1, in0=psv[:, 0:384],
                                    scalar1=vscale[:, tk:tk + 1],
                                    scalar2=None, op0=OP.mult)
            vout = vsb[:, tk, half * 390:half * 390 + 390].rearrange(
                "p (h e) -> p h e", e=HD + 1)[:, :, 0:HD]
            vin = t1.rearrange("p (h d) -> p h d", h=6)
            csvv = csv_t[:, half, :].rearrange("p (h d) -> p h d", h=6)
            nc.vector.scalar_tensor_tensor(out=vout, in0=csvv,
                                           scalar=avcol[:, tk:tk + 1],
                                           in1=vin, op0=OP.mult, op1=OP.add)

    def emit_attnv_pair(p, q, ex2t, ps_y2):
        for j in range(2):
            h = 2 * p + j
            nc.tensor.matmul(ps_y2[j][0:HD + 1, :],
                             vsb[:, 2 * q:2 * q + 2, h * 65:h * 65 + 65],
                             ex2t[:, :, j, :],
                             start=(q == 0), stop=(q == NPAIR - 1),
                             perf_mode=DR)

    def emit_recips(p, ps_y2, pool=None, ptag="mm"):
        pool = pool or ps_mm
        r65s = []
        for j in range(2):
            sr = P4.tile([128, 512], BF16, tag="tf", name=f"sr{p}_{j}")
            nc.vector.tensor_copy(out=sr[HD:HD + 1, :],
                                  in_=ps_y2[j][HD:HD + 1, :])
            psb = pool.tile([64, 512], F32, tag=ptag, name=f"psb{p}_{j}")
            nc.tensor.matmul(psb, onesb[HD:HD + 1, 0:HD], sr[HD:HD + 1, :],
                             start=True, stop=True)
            rbf = P4.tile([128, 512], F32, tag="tf", name=f"rbf{p}_{j}")
            nc.vector.reciprocal_approx_fast(out=rbf[0:HD, :], in_=psb[0:HD, :])
            r65s.append(rbf)
        return r65s

    def emit_deferred_epilogue(p, ps_y2, r65s):
        for j in range(2):
            ps_y = ps_y2[j]
            if j == 0:
                nc.vector.tensor_tensor(y_sb[0:HD, p, :], ps_y[0:HD, :],
                                        r65s[j][0:HD, :], op=OP.mult)
            else:
                yt = P4.tile([128, 512], FP8, tag="yt", name=f"yt{p}")
                nc.vector.tensor_tensor(yt[0:HD, :], ps_y[0:HD, :],
                                        r65s[j][0:HD, :], op=OP.mult)
                nc.sync.dma_start(out=y_sb[HD:128, p, :], in_=yt[0:HD, :])

    y_sb = P1.tile([128, C6, NQ], FP8, tag="y")
    kch = {}
    pend = []
    pend_r = []

    # K(0) tile 0 first (unblocks scores/exp), then deferred key-tile-1
    # stats, then K(0) tile 1; V chunks go inside the p=0 loop
    kch[0] = P2.tile([128, NK], BF16, tag="kch", name="kch0")
    for ci in range(0, C6, 2):
        emit_k_chunk_mm(kch[0], 0, KT[0][0], KT[0][1], ci, 0)
    for t in range(1, NT):
        kstats[t] = emit_stats(xk8, KT[t][0], KT[t][1], f"k{t}", sqeng="gps")
        mb_k[t] = emit_bcast(kstats[t][1], KT[t][1], f"mk{t}")
        emit_kcols(t)
        for ci in range(0, C6, 2):
            emit_k_chunk_mm(kch[0], 0, KT[t][0], KT[t][1], ci, t)
    # csv broadcast tile [128, 2, 384] (negated, x WS host-side)
    csv_t = P1.tile([128, 2, 384], BF16, tag="csvt")
    for half in range(2):
        pmcv = ps_mm.tile([128, 384], F32, tag="mm", name=f"pmcsv{half}")
        nc.tensor.matmul(pmcv, onesb[0:1, :],
                         csv_row[0:1, half * 384:(half + 1) * 384],
                         start=True, stop=True)
        nc.vector.tensor_copy(out=csv_t[:, half, :], in_=pmcv)
    # warm the exp ACT table (after the LN1 Sqrt set, before first exp)
    warme = P1.tile([1, 8], F32, tag="warme")
    nc.scalar.activation(warme, onesb[0:1, 0:8], AF.Exp)
    emit_v_chunk(0)
    emit_v_chunk(1)
    emit_v_chunk(2)
    emit_v_chunk(3)

    for p in range(C6):
        if p < C6 - 1:
            kch[p + 1] = P2.tile([128, NK], BF16, tag="kch", name=f"kch{p + 1}")
            kwork = [(o2, w2, ci, t) for t, (o2, w2) in enumerate(KT)
                     for ci in range(0, C6, 2)]
        else:
            kwork = []
        ex = {}
        ps_y2 = [None, None]
        nextq = 0
        for tk in range(KC):
            pss = ps_sc.tile([128, 2, 512], F32, tag="sc")
            for j in range(2):
                po = j * 64
                nc.tensor.matmul(pss[:, j, :],
                                 kch[p][po:po + 64, ts(tk, 128)],
                                 qT[po:po + 64, p, 0:NQ],
                                 start=True, stop=True)
            if tk % 2 == 0:
                ex[tk // 2] = P6.tile([128, 2, 2, 512], FP8, tag="exp",
                                      name=f"ex_{p}_{tk // 2}", bufs=4)
            nc.scalar.activation(ex[tk // 2][:, tk % 2, :, :], pss, AF.Exp,
                                 scale=expsc[:, tk:tk + 1])
            if p == 0 and 0 <= tk <= KC - 5:
                emit_v_chunk(tk + 4)
            # recips/epilogue of the PREVIOUS p are emitted here, behind this
            # p's first scores, so the PE never stalls on the DVE row chain
            # at the p-boundary (the stall tripped the clock-gate every p)
            if tk == 1 and pend_r:
                pr_ = pend_r.pop()
                r65s = emit_recips(pr_["p"], pr_["ps_y2"])
                pend.append(dict(p=pr_["p"], ps_y2=pr_["ps_y2"], r65s=r65s))
            if tk == 3 and pend:
                emit_deferred_epilogue(**pend.pop())
            if tk == 5:
                ps_y2[0] = ps_mm.tile([128, 512], F32, tag="mm", name=f"psyA{p}")
                ps_y2[1] = ps_mm.tile([128, 512], F32, tag="mm", name=f"psyB{p}")
            if tk >= 5 and (tk - 5) % 2 == 0 and nextq < NPAIR:
                emit_attnv_pair(p, nextq, ex.pop(nextq), ps_y2)
                nextq += 1
            if kwork and p > 0 and tk >= 4:
                o2, w2, ci, t = kwork.pop(0)
                emit_k_chunk_mm(kch[p + 1], p + 1, o2, w2, ci, t)
        while kwork:
            o2, w2, ci, t = kwork.pop(0)
            emit_k_chunk_mm(kch[p + 1], p + 1, o2, w2, ci, t)
        while nextq < NPAIR:
            emit_attnv_pair(p, nextq, ex.pop(nextq), ps_y2)
            nextq += 1
        pend_r.append(dict(p=p, ps_y2=ps_y2))
        if "k" in dbg:
            for (o2, w2) in KT:
                dk_ = P4.tile([128, 512], F32, tag="dbgt", name=f"dbk{p}_{o2}", bufs=1)
                nc.vector.tensor_copy(out=dk_[:, 0:w2], in_=kch[p][:, o2:o2 + w2])
                nc.sync.dma_start(out=dbg["k"][ts(p, 128), o2:o2 + w2],
                                  in_=dk_[:, 0:w2])
    # pre-start uproj + one proj psum group on ready y chunks so the PE
    # keeps streaming while p5's recips wait on the DVE row chain
    ps_us = ps_mm.tile([1, NQ], F32, tag="mm", name="ps_us")
    for c in range(5):
        nc.tensor.matmul(ps_us, uproj_s[:, c:c + 1], y_sb[:, c, :],
                         start=(c == 0), stop=False)
    proj_pre = {}
    psp = ps_mm.tile([128, NQ], F32, tag="mm", name="prj0")
    proj_pre[0] = psp
    for ci in (0, 2):
        nc.tensor.matmul(psp, wproj8[:, ci:ci + 2, ts(0, 128)],
                         y_sb[:, ci:ci + 2, :],
                         start=(ci == 0), stop=False, perf_mode=DR)
    pr_ = pend_r.pop()
    r65s = emit_recips(pr_["p"], pr_["ps_y2"], pool=ps_sc, ptag="sc")
    emit_deferred_epilogue(p=pr_["p"], ps_y2=pr_["ps_y2"], r65s=r65s)
    nc.tensor.matmul(ps_us, uproj_s[:, 5:6], y_sb[:, 5, :],
                     start=False, stop=True)
    if "y" in dbg:
        for c in range(C6):
            dy_ = P4.tile([128, 512], F32, tag="dbgt", name=f"dby{c}", bufs=1)
            nc.vector.tensor_copy(out=dy_, in_=y_sb[:, c, :])
            nc.sync.dma_start(out=dbg["y"][ts(c, 128), :], in_=dy_)

    # ========== proj (fp8 DR, PE-centered) -> d2 = x2 - mean2 ==========
    d2_sb = P1.tile([128, C6, NQ], F32, tag="x2")
    # LN2 mean rows (need only u.y + sumx0)
    srow = P6.tile([1, NQ], F32, tag="rowf", bufs=4, name="srow2")
    nc.vector.scalar_tensor_tensor(out=srow, in0=ps_us, scalar=float(sbp),
                                   in1=sumx0, op0=OP.add, op1=OP.add)
    mrow2n = P6.tile([1, NQ], BF16, tag="row", bufs=6, name="mrow2n")
    nc.vector.tensor_scalar(out=mrow2n, in0=srow, scalar1=-WS / D,
                            scalar2=None, op0=OP.mult)
    m2row = P6.tile([1, NQ], BF16, tag="row", bufs=6, name="m2row")
    nc.vector.tensor_scalar(out=m2row, in0=srow, scalar1=1.0 / D,
                            scalar2=None, op0=OP.mult)

    sq2 = P4.tile([128, C6, NQ], FP8, tag="sq", name="sq2", bufs=2)
    for co in range(C6):
        if co in proj_pre:
            ps = proj_pre[co]
            nc.tensor.matmul(ps, wproj8[:, 4:6, ts(co, 128)],
                             y_sb[:, 4:6, :],
                             start=False, stop=False, perf_mode=DR)
        else:
            ps = ps_mm.tile([128, NQ], F32, tag="mm")
            for ci in range(0, C6, 2):
                nc.tensor.matmul(ps, wproj8[:, ci:ci + 2, ts(co, 128)],
                                 y_sb[:, ci:ci + 2, :],
                                 start=(ci == 0), stop=False, perf_mode=DR)
        # center in-psum: ps += 1 (x) (-WS m2)
        nc.tensor.matmul(ps, onesb[0:1, :], mrow2n, start=False, stop=True)
        nc.vector.scalar_tensor_tensor(out=d2_sb[:, co, :], in0=ps,
                                       scalar=1.0 / WS,
                                       in1=xb_sb[:, co, :],
                                       op0=OP.mult, op1=OP.add)
        nc.scalar.activation(sq2[:, co, :], d2_sb[:, co, :], AF.Square)

    if "x2" in dbg:
        for c in range(C6):
            dx2_ = P4.tile([128, 512], F32, tag="dbgt", name=f"dbx2{c}", bufs=1)
            nc.vector.tensor_copy(out=dx2_, in_=d2_sb[:, c, :])
            nc.sync.dma_start(out=dbg["x2"][ts(c, 128), :], in_=dx2_)

    # LN2 var + rstd (var = E[d2^2] exactly -- d2 already centered)
    ps_sq2 = ps_mm.tile([1, NQ], F32, tag="mm", name="ps_sq2")
    for ci in range(0, C6, 2):
        nc.tensor.matmul(ps_sq2, ones8, sq2[:, ci:ci + 2, :],
                         start=(ci == 0), stop=(ci == C6 - 2), perf_mode=DR)
    vrow2 = P6.tile([1, NQ], F32, tag="rowf", bufs=4, name="vrow2")
    nc.vector.tensor_scalar(out=vrow2, in0=ps_sq2, scalar1=1.0 / D,
                            scalar2=None, op0=OP.mult)
    srt2 = P6.tile([1, NQ], F32, tag="rowf", bufs=4, name="srt2")
    nc.scalar.activation(srt2, vrow2, AF.Sqrt, bias=eps1)
    # warm the gelu table while PE runs proj/fc1 lead-in
    warmg = P6.tile([1, 8], F32, tag="rowf", bufs=4, name="warmg")
    nc.scalar.activation(warmg, srt2[0:1, 0:8], AF.Gelu)
    rf2 = P6.tile([1, NQ], F32, tag="rowf", bufs=4, name="rf2")
    nc.vector.reciprocal_approx_fast(out=rf2, in_=srt2)
    rrow2 = P6.tile([1, NQ], BF16, tag="row", bufs=6, name="rrow2")
    nc.vector.tensor_copy(out=rrow2, in_=rf2)
    prb2 = ps_mm.tile([128, NQ], F32, tag="mm", name="prb2")
    nc.tensor.matmul(prb2, onesb[0:1, :], rrow2, start=True, stop=True)
    xn2 = P2.tile([128, C6, NQ], FP8, tag="xn2", bufs=1)
    for c in range(C6):
        nc.vector.tensor_tensor(xn2[:, c, :], d2_sb[:, c, :], prb2,
                                op=OP.mult)

    if "xn2" in dbg:
        for c in range(C6):
            dn_ = P4.tile([128, 512], F32, tag="dbgt", name=f"dbn{c}", bufs=1)
            nc.vector.tensor_copy(out=dn_, in_=xn2[:, c, :])
            nc.sync.dma_start(out=dbg["xn2"][ts(c, 128), :], in_=dn_)

    # ========== MLP: fc1 fp8 DR -> gelu; fc2 bf16 ==========
    h_sb = P2.tile([128, HO24, NQ], BF16, tag="h", bufs=1)
    wfc2b = None
    for ho in range(HO24):
        if ho == 12:
            wfc2b = P2.tile([128, 12, D], BF16, tag="wbig", bufs=2)
            nc.sync.dma_start(
                out=wfc2b,
                in_=wfc2_d.rearrange("(a p) o -> p a o", p=128)[:, 12:24, :])
        ps = ps_mm.tile([128, NQ], F32, tag="mm")
        for ci in range(0, C6, 2):
            nc.tensor.matmul(ps, wfc1_sb[:, ci:ci + 2, ts(ho, 128)],
                             xn2[:, ci:ci + 2, :],
                             start=(ci == 0), stop=(ci == C6 - 2), perf_mode=DR)
        nc.scalar.activation(h_sb[:, ho, :], ps, AF.Gelu,
                             bias=bfc1_s[:, ho:ho + 1], scale=1.0 / WS)
    for co in range(C6):
        ps = ps_mm.tile([128, NQ], F32, tag="mm")
        for ho in range(HO24):
            wsrc = wfc2a if ho < 12 else wfc2b
            nc.tensor.matmul(ps, wsrc[:, ho % 12, ts(co, 128)], h_sb[:, ho, :],
                             start=(ho == 0), stop=False)
        # de-center the residual: ps += 1 (x) m2 (out = d2 + m2 + mlp)
        nc.tensor.matmul(ps, onesb[0:1, :], m2row, start=False, stop=True)
        o = P2.tile([128, NQ], F32, tag="ot")
        nc.vector.scalar_tensor_tensor(out=o, in0=ps,
                                       scalar=bfc2_s[:, co:co + 1],
                                       in1=d2_sb[:, co, :],
                                       op0=OP.add, op1=OP.add)
        nc.sync.dma_start(out=out_d[ts(co, 128), :], in_=o)

    for cm in (ps_sc_cm, ps_mm_cm, P6_cm, P4_cm, P2_cm, P1_cm):
        cm.__exit__(None, None, None)


def _host_prep(x, mask, ln1_g, ln1_b, qkv_w, proj_w, proj_b, ln2_g, ln2_b,
               fc1_w, fc1_b, fc2_w, fc2_b):
    bf = ml_dtypes.bfloat16
    fp8 = ml_dtypes.float8_e4m3
    f32 = np.float32
    x = np.asarray(x, f32)
    mask = np.asarray(mask)
    qkv_w = np.asarray(qkv_w, f32)
    proj_w = np.asarray(proj_w, f32)
    fc1_w = np.asarray(fc1_w, f32)
    fc2_w = np.asarray(fc2_w, f32)
    ln1_g = np.asarray(ln1_g, f32); ln1_b = np.asarray(ln1_b, f32)
    ln2_g = np.asarray(ln2_g, f32); ln2_b = np.asarray(ln2_b, f32)
    proj_b = np.asarray(proj_b, f32)
    fc1_b = np.asarray(fc1_b, f32); fc2_b = np.asarray(fc2_b, f32)

    # the raw-G scores fold (rstd on the exp scale) needs zero qkv bias,
    # which holds iff ln1_b == 0 (the reference qkv matmul has no bias)
    assert np.all(ln1_b == 0.0), "kernel assumes ln1_b == 0"

    wqkv_f = qkv_w * ln1_g[None, :]
    wqkv_f[0:D] *= SCALE
    bproj_f = proj_b
    cs = wqkv_f.sum(axis=1)          # [2304] per-output-channel colsums
    wfc1_f = fc1_w * ln2_g[None, :]
    bfc1_f = fc1_w @ ln2_b + fc1_b

    shared = {
        "wqkv": np.ascontiguousarray(wqkv_f.T * WS).astype(fp8),
        "wproj": np.ascontiguousarray(proj_w.T * WS).astype(fp8),
        "wfc1": np.ascontiguousarray(wfc1_f.T * WS).astype(fp8),
        "wfc2": np.ascontiguousarray(fc2_w.T).astype(bf),
        "uproj": np.ascontiguousarray(
            proj_w.sum(axis=0).reshape(6, 128).T).astype(bf),
        "csrows": np.ascontiguousarray((-WS * cs)[None, :]).astype(bf),
    }
    sbp = float(bproj_f.sum())

    # compacted unmasked keys per batch
    kidx = [np.where(mask[b] != 1)[0] for b in range(B)]
    nk_max = max(len(i) for i in kidx)
    NK = max(((nk_max + 255) // 256) * 256, 512)

    # packed f32 constants: m01 (per-core) | csq | csk | bfc1 | bfc2
    constf_tail = np.concatenate([
        (-WS * cs[0:D]).reshape(6, 128).T,
        (-WS * cs[D:2 * D]).reshape(6, 128).T,
        bfc1_f.reshape(24, 128).T,
        fc2_b.reshape(6, 128).T,
    ], axis=1).astype(f32)

    in_maps = []
    for core in range(NC):
        b, s = divmod(core, NSH)
        xq = x[b][s * NQ:(s + 1) * NQ]       # [NQ, D]
        nk = len(kidx[b])
        xk = np.zeros((NK, D), f32)
        xk[:nk] = x[b][kidx[b]]
        m01k = np.zeros((NK,), f32)
        m01k[:nk] = 1.0
        im = dict(shared)
        im["xq8"] = np.ascontiguousarray(xq.T).astype(fp8)
        im["xk8"] = np.ascontiguousarray(xk.T).astype(fp8)
        im["xb"] = np.ascontiguousarray(xq.T + bproj_f[:, None]).astype(bf)
        im["constf"] = np.ascontiguousarray(np.concatenate(
            [m01k.reshape(NK // 128, 128).T.astype(f32), constf_tail],
            axis=1))
        in_maps.append(im)
    return in_maps, sbp, NK


def kernel(**inputs):
    in_maps, sbp, NK = _host_prep(**inputs)
    if _cached.get("key") != (sbp, NK):
        _cached["nc"] = _build_nc(sbp, NK)
        _cached["key"] = (sbp, NK)
    res = run_bass_kernel_spmd(_cached["nc"], in_maps, core_ids=list(range(NC)))
    out = np.empty((B, N, D), np.float32)
    for core in range(NC):
        b, s = divmod(core, NSH)
        out[b, s * NQ:(s + 1) * NQ, :] = res.results[core]["out"].T
    return out


# revision 31
# speedup vs baseline: 1.0118x; 1.0118x over previous
"""Trainium2 Bass kernel for a dense transformer block (pre-LN, MHA + GELU MLP).

Problem shapes (hardcoded): x [2, 2048, 768] f32, mask [2, 2048] int32,
12 heads x 64 dims, hidden 3072.

Sharding: 8 cores = (batch b in {0,1}) x (token shard s in {0..3}).
Each core handles its 512-query shard of Q / attention rows / MLP / output.

Key compaction: the key-padding mask kills ~half the keys, so the host
gathers only the unmasked keys' x columns into xk [D, NK] (NK = count
rounded up to 256, zero-padded). Pad keys are neutralized by zeroing their
V rows and ones-column entries.

LN1 is never applied to activations ("raw-G" trick): Q/K/V matmuls run on
raw fp8 x directly (so the PE starts as soon as DMA lands, with no
LN-apply serialization), and the normalization is folded into the
epilogues via per-output-channel weight column sums:
    q[o,t] = r[t] * (G[o,t] - m[t]*colsum_w[o])        (valid: ln1_b == 0)
The query-side rstd folds into the Q epilogue; the key-side rstd folds
into the exp's per-partition ACT scale (keys sit on PSUM partitions for
scores), and the V-side mean/rstd fold into the V epilogue using
transposed per-key mean/rstd columns (tiny PE transpose matmuls).

On-chip layout is feature-major: activations are [features, tokens],
every matmul contracts over the partition dim. Per-token LN stats come
from ones-vector DoubleRow matmuls on raw fp8 x; the fp8 squares are
split across DVE/ACT/GpSimd so no one engine serializes the startup.
Softmax denominators come free from a ones column appended to V (M=65
matmul). Max-subtraction is skipped: |scores| <= ~4 by construction.
LN2 mean is folded into the proj PSUM as a K=1 rank-1 accumulate (and
un-folded in the fc2 PSUM), so the residual stream lives centered (d2)
and the LN2 apply is one multiply.

fp8 DoubleRow everywhere except fc2 (kept bf16: quantizing the 3072-wide
contraction would eat too much of the 2e-2 error budget). DMAs are one
issue per tensor (descriptor issue costs ~600ns on the Sync engine) in
consumer-priority order. Perf note for this box: the PE runs at 1.2 GHz
through dependency-mixed regions (only long uniform MM streams reach
2.4 GHz), so attention-path epilogues belong on DVE, not on the PE.
"""

import numpy as np
import ml_dtypes

import concourse.bass as bass
import concourse.tile as tile
import concourse.mybir as mybir
from concourse import bacc
from concourse.bass import ts
from concourse.bass_utils import run_bass_kernel_spmd
from concourse.alu_op_type import AluOpType

BF16 = mybir.dt.bfloat16
F32 = mybir.dt.float32
FP8 = mybir.dt.float8e4
DR = mybir.MatmulPerfMode.DoubleRow
WS = 32.0   # fp8 weight scale (dodges e4m3 subnormals)

B = 2
N = 2048
D = 768
H = 12
HD = 64
HID = 3072
EPS = 1e-5
SCALE = HD ** -0.5
NQ = 512          # queries per core
NSH = N // NQ     # token shards per batch
NC = B * NSH      # 8 cores
C6 = D // 128     # feature chunks
HO24 = HID // 128

AF = mybir.ActivationFunctionType
OP = AluOpType

_cached = {}
_rid = [0]


def _rid_next():
    _rid[0] += 1
    return _rid[0]


def _build_nc(sbp, NK):
    nc = bacc.Bacc("TRN2", target_bir_lowering=False, debug=False,
                   enable_asserts=False, num_devices=NC)

    xq8 = nc.dram_tensor("xq8", [D, NQ], FP8, kind="ExternalInput").ap()
    xk8 = nc.dram_tensor("xk8", [D, NK], FP8, kind="ExternalInput").ap()
    xb = nc.dram_tensor("xb", [D, NQ], BF16, kind="ExternalInput").ap()
    wqkv = nc.dram_tensor("wqkv", [D, 3 * D], FP8, kind="ExternalInput").ap()
    wproj = nc.dram_tensor("wproj", [D, D], FP8, kind="ExternalInput").ap()
    wfc1 = nc.dram_tensor("wfc1", [D, HID], FP8, kind="ExternalInput").ap()
    wfc2 = nc.dram_tensor("wfc2", [HID, D], BF16, kind="ExternalInput").ap()
    uproj = nc.dram_tensor("uproj", [128, 6], BF16, kind="ExternalInput").ap()
    # negated WS-scaled weight column sums as rows: csq | csk | csv
    csrows = nc.dram_tensor("csrows", [1, 3 * D], BF16,
                            kind="ExternalInput").ap()
    KC = NK // 128
    # packed f32 constants: m01 | csq | csk | bfc1 | bfc2
    constf = nc.dram_tensor("constf", [128, KC + 42], F32,
                            kind="ExternalInput").ap()
    out_d = nc.dram_tensor("out", [D, NQ], F32, kind="ExternalOutput").ap()
    import os
    dbg = {}
    if os.environ.get("KDBG"):
        dbg["y"] = nc.dram_tensor("dbg_y", [D, NQ], F32, kind="ExternalOutput").ap()
        dbg["x2"] = nc.dram_tensor("dbg_x2", [D, NQ], F32, kind="ExternalOutput").ap()
        dbg["q"] = nc.dram_tensor("dbg_q", [D, NQ], F32, kind="ExternalOutput").ap()
        dbg["k"] = nc.dram_tensor("dbg_k", [D, NK], F32, kind="ExternalOutput").ap()
        dbg["xn2"] = nc.dram_tensor("dbg_xn2", [D, NQ], F32, kind="ExternalOutput").ap()

    with tile.TileContext(nc) as tc:
        _body(nc, tc, sbp, NK, xq8, xk8, xb, wqkv, wproj, wfc1, wfc2,
              uproj, csrows, constf, out_d, dbg)
    nc.compile()
    return nc


def _body(nc, tc, sbp, NK, xq8_d, xk8_d, xb_d, wqkv_d, wproj_d, wfc1_d,
          wfc2_d, uproj_d, csrows_d, constf_d, out_d, dbg=None):
    dbg = dbg or {}
    KC = NK // 128
    NPAIR = KC // 2
    KT = []
    off = 0
    while off < NK:
        w = min(512, NK - off)
        KT.append((off, w))
        off += w
    NT = len(KT)

    P1_cm = tc.tile_pool(name="p1", bufs=1); P1 = P1_cm.__enter__()
    P2_cm = tc.tile_pool(name="p2", bufs=2); P2 = P2_cm.__enter__()
    P4_cm = tc.tile_pool(name="p4", bufs=4); P4 = P4_cm.__enter__()
    P6_cm = tc.tile_pool(name="p6", bufs=6); P6 = P6_cm.__enter__()
    ps_mm_cm = tc.tile_pool(name="ps_mm", bufs=4, space="PSUM")
    ps_mm = ps_mm_cm.__enter__()
    ps_sc_cm = tc.tile_pool(name="ps_sc", bufs=2, space="PSUM")
    ps_sc = ps_sc_cm.__enter__()

    # ---- DMAs: one issue per tensor (issue cost ~600ns each on Sync),
    # ordered by when the consumer needs the data ----
    xq8_r = xq8_d.rearrange("(a p) t -> p a t", p=128)
    xk8_r = xk8_d.rearrange("(a p) t -> p a t", p=128)
    wqkv_r = wqkv_d.rearrange("(a p) o -> p a o", p=128)
    x8 = P1.tile([128, C6, NQ], FP8, tag="x8")
    nc.sync.dma_start(out=x8, in_=xq8_r)
    w_sb = P2.tile([128, C6, 3 * D], FP8, tag="wbig", bufs=2)
    nc.sync.dma_start(out=w_sb[:, :, 0:D], in_=wqkv_r[:, :, 0:D])
    xk8 = P1.tile([128, C6, NK], FP8, tag="xk8")
    for (o2, w2) in KT:
        nc.sync.dma_start(out=xk8[:, :, o2:o2 + w2], in_=xk8_r[:, :, o2:o2 + w2])
    # packed f32 constants: m01 | csq | csk | bfc1 | bfc2
    constf = P1.tile([128, KC + 42], F32, tag="constf")
    nc.sync.dma_start(out=constf, in_=constf_d)
    m01 = constf[:, 0:KC]
    csq_s = constf[:, KC:KC + 6]
    csk_s = constf[:, KC + 6:KC + 12]
    bfc1_s = constf[:, KC + 12:KC + 36]
    bfc2_s = constf[:, KC + 36:KC + 42]
    nc.sync.dma_start(out=w_sb[:, :, D:2 * D], in_=wqkv_r[:, :, D:2 * D])
    csr = P1.tile([1, 3 * D], BF16, tag="csr")
    nc.sync.dma_start(out=csr, in_=csrows_d)
    csq_row = csr[:, 0:D]
    csk_row = csr[:, D:2 * D]
    csv_row = csr[:, 2 * D:3 * D]
    nc.sync.dma_start(out=w_sb[:, :, 2 * D:3 * D], in_=wqkv_r[:, :, 2 * D:3 * D])
    uproj_s = P1.tile([128, 6], BF16, tag="uproj")
    nc.sync.dma_start(out=uproj_s, in_=uproj_d)

    # ---- on-chip constants ----
    onesb = P1.tile([128, 128], BF16, tag="onesb")
    nc.vector.memset(onesb, 1.0)
    ones8_t = P1.tile([128, 2, 16], FP8, tag="ones8")
    nc.vector.memset(ones8_t, 1.0)
    ones8 = ones8_t[:, :, 0:1]
    eps1 = P1.tile([1, 1], F32, tag="eps1")
    nc.vector.memset(eps1, EPS)
    sumx0 = P1.tile([1, NQ], BF16, tag="sumx0")

    # ---- later-phase DMAs (behind the attention-critical ones) ----
    xb_sb = P1.tile([128, C6, NQ], BF16, tag="xb")
    nc.sync.dma_start(out=xb_sb, in_=xb_d.rearrange("(a p) t -> p a t", p=128))
    wproj8 = P1.tile([128, C6, D], FP8, tag="wpj")
    nc.sync.dma_start(out=wproj8, in_=wproj_d.rearrange("(a p) o -> p a o", p=128))
    wfc1_sb = P1.tile([128, C6, HID], FP8, tag="wfc1")
    nc.sync.dma_start(out=wfc1_sb, in_=wfc1_d.rearrange("(a p) o -> p a o", p=128))
    wfc2a = P2.tile([128, 12, D], BF16, tag="wbig", bufs=2)
    nc.sync.dma_start(out=wfc2a,
                      in_=wfc2_d.rearrange("(a p) o -> p a o", p=128)[:, 0:12, :])

    # ========== LN stats on raw fp8 x (sum/sumsq via DR matmuls) ==========
    def emit_stats(src, t_off, w, nm, sqeng="dve"):
        i = _rid_next()
        ps_sum = ps_mm.tile([1, w], F32, tag="mm", name=f"pssum{i}")
        ps_sq = ps_mm.tile([1, w], F32, tag="mm", name=f"pssq{i}")
        for ci in range(0, C6, 2):
            nc.tensor.matmul(ps_sum, ones8, src[:, ci:ci + 2, t_off:t_off + w],
                             start=(ci == 0), stop=(ci == C6 - 2), perf_mode=DR)
        sq = P4.tile([128, C6, w], FP8, tag="sq", name=f"sq{i}", bufs=2)
        for ci in range(0, C6, 2):
            if sqeng == "act":
                nc.scalar.activation(sq[:, ci:ci + 2, :],
                                     src[:, ci:ci + 2, t_off:t_off + w],
                                     AF.Square)
            elif sqeng == "gps":
                nc.gpsimd.tensor_tensor(sq[:, ci:ci + 2, :],
                                        src[:, ci:ci + 2, t_off:t_off + w],
                                        src[:, ci:ci + 2, t_off:t_off + w],
                                        op=OP.mult)
            else:
                nc.vector.tensor_tensor(sq[:, ci:ci + 2, :],
                                        src[:, ci:ci + 2, t_off:t_off + w],
                                        src[:, ci:ci + 2, t_off:t_off + w],
                                        op=OP.mult)
        for ci in range(0, C6, 2):
            nc.tensor.matmul(ps_sq, ones8, sq[:, ci:ci + 2, :],
                             start=(ci == 0), stop=(ci == C6 - 2), perf_mode=DR)
        mrow = P6.tile([1, w], BF16, tag="row", bufs=6, name=f"mrow{i}")
        nc.vector.tensor_scalar(out=mrow, in0=ps_sum, scalar1=1.0 / D,
                                scalar2=None, op0=OP.mult)
        m2 = P6.tile([1, w], F32, tag="rowf", bufs=4, name=f"m2_{i}")
        nc.vector.tensor_tensor(m2, mrow, mrow, op=OP.mult)
        vrow = P6.tile([1, w], F32, tag="rowf", bufs=4, name=f"vrow{i}")
        nc.vector.scalar_tensor_tensor(out=vrow, in0=ps_sq, scalar=1.0 / D,
                                       in1=m2, op0=OP.mult, op1=OP.subtract)
        srt = P6.tile([1, w], F32, tag="rowf", bufs=4, name=f"srt{i}")
        nc.scalar.activation(srt, vrow, AF.Sqrt, bias=eps1)
        rf = P6.tile([1, w], F32, tag="rowf", bufs=4, name=f"rf{i}")
        nc.vector.reciprocal_approx_fast(out=rf, in_=srt)
        rrow = P6.tile([1, w], BF16, tag="row", bufs=6, name=f"rrow{i}")
        nc.vector.tensor_copy(out=rrow, in_=rf)
        return ps_sum, mrow, rrow

    # ---- broadcast tiles (PE K=1 matmul + copy) ----
    def emit_bcast(row, w, nm):
        pmb = ps_mm.tile([128, w], F32, tag="mm", name=f"pmb_{nm}")
        nc.tensor.matmul(pmb, onesb[0:1, :], row, start=True, stop=True)
        t = P4.tile([128, w], BF16, tag="bcs", name=f"bc_{nm}", bufs=4)
        nc.vector.tensor_copy(out=t, in_=pmb)
        return t

    # ---- per-key mean/rstd columns via tiny PE transposes ----
    mkc = P1.tile([128, KC], F32, tag="mkc")
    expsc = P1.tile([128, KC], F32, tag="expsc")
    vscale = P1.tile([128, KC], F32, tag="vscale")
    avcol = P1.tile([128, KC], F32, tag="avcol")

    def emit_kcols(t):
        o2, w2 = KT[t]
        nch = w2 // 128
        c0 = o2 // 128
        i = _rid_next()
        pm = ps_mm.tile([128, nch], F32, tag="mm", name=f"pstm{i}")
        pr = ps_mm.tile([128, nch], F32, tag="mm", name=f"pstr{i}")
        for k in range(nch):
            nc.tensor.matmul(pm[:, k:k + 1],
                             kstats[t][1][0:1, k * 128:(k + 1) * 128],
                             onesb[0:1, 0:1], start=True, stop=True)
            nc.tensor.matmul(pr[:, k:k + 1],
                             kstats[t][2][0:1, k * 128:(k + 1) * 128],
                             onesb[0:1, 0:1], start=True, stop=True)
        nc.vector.tensor_copy(out=mkc[:, c0:c0 + nch], in_=pm)
        nc.vector.tensor_scalar(out=expsc[:, c0:c0 + nch], in0=pr,
                                scalar1=1.0 / WS, scalar2=None, op0=OP.mult)
        nc.vector.tensor_tensor(vscale[:, c0:c0 + nch], expsc[:, c0:c0 + nch],
                                m01[:, c0:c0 + nch], op=OP.mult)
        nc.vector.tensor_tensor(avcol[:, c0:c0 + nch], mkc[:, c0:c0 + nch],
                                vscale[:, c0:c0 + nch], op=OP.mult)

    # stats: queries, then key tile 0 (tile 1 deferred until after the
    # first-scores dependencies so exp can start ASAP)
    ps_sum_q, mrow_q, rrow_q = emit_stats(x8, 0, NQ, "q")
    nc.vector.tensor_copy(out=sumx0, in_=ps_sum_q)
    kstats = [None] * NT
    kstats[0] = emit_stats(xk8, KT[0][0], KT[0][1], "k0", sqeng="act")
    mb_k = [None] * NT
    mb_k[0] = emit_bcast(kstats[0][1], KT[0][1], "mk0")
    emit_kcols(0)
    mb_q = emit_bcast(mrow_q, NQ, "mq")
    rbrow = P6.tile([1, NQ], BF16, tag="row", bufs=6, name="rbrow")
    nc.vector.tensor_scalar(out=rbrow, in0=rrow_q, scalar1=1.0 / WS,
                            scalar2=None, op0=OP.mult)
    rb_q = emit_bcast(rbrow, NQ, "rq")

    # ========== Q (raw-G + folded epilogue) ==========
    qT = P2.tile([128, C6, NQ], BF16, tag="qT", bufs=1)
    for co in range(C6):
        ps = ps_mm.tile([128, NQ], F32, tag="mm")
        for ci in range(0, C6, 2):
            nc.tensor.matmul(ps, w_sb[:, ci:ci + 2, ts(co, 128)],
                             x8[:, ci:ci + 2, :],
                             start=(ci == 0), stop=(ci == C6 - 2), perf_mode=DR)
        t1 = P4.tile([128, NQ], BF16, tag="tmp", name=f"qt1_{co}")
        nc.vector.scalar_tensor_tensor(out=t1, in0=mb_q,
                                       scalar=csq_s[:, co:co + 1], in1=ps,
                                       op0=OP.mult, op1=OP.add)
        nc.vector.tensor_tensor(qT[:, co, :], t1, rb_q, op=OP.mult)

    if "q" in dbg:
        for c in range(C6):
            dq_ = P4.tile([128, 512], F32, tag="dbgt", name=f"dbq{c}", bufs=1)
            nc.vector.tensor_copy(out=dq_, in_=qT[:, c, :])
            nc.sync.dma_start(out=dbg["q"][ts(c, 128), :], in_=dq_)

    # ========== attention pipeline ==========
    vsb = P1.tile([128, KC, 16 * ((H * (HD + 1) + 15) // 16)], FP8, tag="vsb")
    m01r = bass.AP(tensor=m01.tensor, offset=m01.offset,
                   ap=[list(m01.ap[0]), list(m01.ap[1]), [0, H], [0, 1]])
    vsb_h = vsb[:, :, 0:H * (HD + 1)].rearrange("p k (h e) -> p k h e", e=HD + 1)
    nc.vector.tensor_copy(out=vsb_h[:, :, :, HD:HD + 1], in_=m01r)

    kch_state = [None]

    def emit_k_chunk_mm(kch_p, p, o2, w2, ci, t):
        if ci == 0:
            kst = ps_mm.tile([128, w2], F32, tag="mm", name=f"kst{_rid_next()}")
            kch_state[0] = kst
        nc.tensor.matmul(kch_state[0], w_sb[:, ci:ci + 2, ts(6 + p, 128)],
                         xk8[:, ci:ci + 2, o2:o2 + w2],
                         start=(ci == 0), stop=(ci == C6 - 2), perf_mode=DR)
        if ci == C6 - 2:
            nc.vector.scalar_tensor_tensor(out=kch_p[:, o2:o2 + w2],
                                           in0=mb_k[t],
                                           scalar=csk_s[:, p:p + 1],
                                           in1=kch_state[0],
                                           op0=OP.mult, op1=OP.add)
            kch_state[0] = None

    def emit_v_chunk(tk):
        i = _rid_next()
        for half in range(2):
            psv = ps_mm.tile([128, 512], F32, tag="mm", name=f"psv{tk}_{half}")
            for ci in range(0, C6, 2):
                nc.tensor.matmul(psv[:, 0:384],
                                 xk8[:, ci:ci + 2, ts(tk, 128)],
                                 w_sb[:, ci:ci + 2, 12 * 128 + half * 384:
                                      12 * 128 + (half + 1) * 384],
                                 start=(ci == 0), stop=(ci == C6 - 2),
                                 perf_mode=DR)
            t1 = P4.tile([128, 384], BF16, tag="vt", name=f"vt{i}_{half}",
                         bufs=4)
            nc.vector.tensor_scalar(out=t1, in0=psv[:, 0:384],
                                    scalar1=vscale[:, tk:tk + 1],
                                    scalar2=None, op0=OP.mult)
            vout = vsb[:, tk, half * 390:half * 390 + 390].rearrange(
                "p (h e) -> p h e", e=HD + 1)[:, :, 0:HD]
            vin = t1.rearrange("p (h d) -> p h d", h=6)
            csvv = csv_t[:, half, :].rearrange("p (h d) -> p h d", h=6)
            nc.vector.scalar_tensor_tensor(out=vout, in0=csvv,
                                           scalar=avcol[:, tk:tk + 1],
                                           in1=vin, op0=OP.mult, op1=OP.add)

    def emit_attnv_pair(p, q, ex2t, ps_y2):
        for j in range(2):
            h = 2 * p + j
            nc.tensor.matmul(ps_y2[j][0:HD + 1, :],
                             vsb[:, 2 * q:2 * q + 2, h * 65:h * 65 + 65],
                             ex2t[:, :, j, :],
                             start=(q == 0), stop=(q == NPAIR - 1),
                             perf_mode=DR)

    def emit_recips(p, ps_y2, pool=None, ptag="mm"):
        pool = pool or ps_mm
        r65s = []
        for j in range(2):
            sr = P4.tile([128, 512], BF16, tag="tf", name=f"sr{p}_{j}")
            nc.vector.tensor_copy(out=sr[HD:HD + 1, :],
                                  in_=ps_y2[j][HD:HD + 1, :])
            psb = pool.tile([64, 512], F32, tag=ptag, name=f"psb{p}_{j}")
            nc.tensor.matmul(psb, onesb[HD:HD + 1, 0:HD], sr[HD:HD + 1, :],
                             start=True, stop=True)
            rbf = P4.tile([128, 512], F32, tag="tf", name=f"rbf{p}_{j}")
            nc.vector.reciprocal_approx_fast(out=rbf[0:HD, :], in_=psb[0:HD, :])
            r65s.append(rbf)
        return r65s

    def emit_deferred_epilogue(p, ps_y2, r65s):
        for j in range(2):
            ps_y = ps_y2[j]
            if j == 0:
                nc.vector.tensor_tensor(y_sb[0:HD, p, :], ps_y[0:HD, :],
                                        r65s[j][0:HD, :], op=OP.mult)
            else:
                yt = P4.tile([128, 512], FP8, tag="yt", name=f"yt{p}")
                nc.vector.tensor_tensor(yt[0:HD, :], ps_y[0:HD, :],
                                        r65s[j][0:HD, :], op=OP.mult)
                nc.sync.dma_start(out=y_sb[HD:128, p, :], in_=yt[0:HD, :])

    y_sb = P1.tile([128, C6, NQ], FP8, tag="y")
    kch = {}
    pend = []
    pend_r = []

    # K(0) tile 0 first (unblocks scores/exp), then deferred key-tile-1
    # stats, then K(0) tile 1; V chunks go inside the p=0 loop
    kch[0] = P2.tile([128, NK], BF16, tag="kch", name="kch0")
    for ci in range(0, C6, 2):
        emit_k_chunk_mm(kch[0], 0, KT[0][0], KT[0][1], ci, 0)
    for t in range(1, NT):
        kstats[t] = emit_stats(xk8, KT[t][0], KT[t][1], f"k{t}", sqeng="gps")
        mb_k[t] = emit_bcast(kstats[t][1], KT[t][1], f"mk{t}")
        emit_kcols(t)
        for ci in range(0, C6, 2):
            emit_k_chunk_mm(kch[0], 0, KT[t][0], KT[t][1], ci, t)
    # csv broadcast tile [128, 2, 384] (negated, x WS host-side)
    csv_t = P1.tile([128, 2, 384], BF16, tag="csvt")
    for half in range(2):
        pmcv = ps_mm.tile([128, 384], F32, tag="mm", name=f"pmcsv{half}")
        nc.tensor.matmul(pmcv, onesb[0:1, :],
                         csv_row[0:1, half * 384:(half + 1) * 384],
                         start=True, stop=True)
        nc.vector.tensor_copy(out=csv_t[:, half, :], in_=pmcv)
    # warm the exp ACT table (after the LN1 Sqrt set, before first exp)
    warme = P1.tile([1, 8], F32, tag="warme")
    nc.scalar.activation(warme, onesb[0:1, 0:8], AF.Exp)
    emit_v_chunk(0)
    emit_v_chunk(1)
    emit_v_chunk(2)
    emit_v_chunk(3)

    for p in range(C6):
        if p < C6 - 1:
            kch[p + 1] = P2.tile([128, NK], BF16, tag="kch", name=f"kch{p + 1}")
            kwork = [(o2, w2, ci, t) for t, (o2, w2) in enumerate(KT)
                     for ci in range(0, C6, 2)]
        else:
            kwork = []
        ex = {}
        ps_y2 = [None, None]
        nextq = 0
        for tk in range(KC):
            pss = ps_sc.tile([128, 2, 512], F32, tag="sc")
            for j in range(2):
                po = j * 64
                nc.tensor.matmul(pss[:, j, :],
                                 kch[p][po:po + 64, ts(tk, 128)],
                                 qT[po:po + 64, p, 0:NQ],
                                 start=True, stop=True)
            if tk % 2 == 0:
                ex[tk // 2] = P6.tile([128, 2, 2, 512], FP8, tag="exp",
                                      name=f"ex_{p}_{tk // 2}", bufs=4)
            nc.scalar.activation(ex[tk // 2][:, tk % 2, :, :], pss, AF.Exp,
                                 scale=expsc[:, tk:tk + 1])
            if p == 0 and 0 <= tk <= KC - 5:
                emit_v_chunk(tk + 4)
            # recips/epilogue of the PREVIOUS p are emitted here, behind this
            # p's first scores, so the PE never stalls on the DVE row chain
            # at the p-boundary (the stall tripped the clock-gate every p)
            if tk == 1 and pend_r:
                pr_ = pend_r.pop()
                r65s = emit_recips(pr_["p"], pr_["ps_y2"])
                pend.append(dict(p=pr_["p"], ps_y2=pr_["ps_y2"], r65s=r65s))
            if tk == 3 and pend:
                emit_deferred_epilogue(**pend.pop())
            if tk == 5:
                ps_y2[0] = ps_mm.tile([128, 512], F32, tag="mm", name=f"psyA{p}")
                ps_y2[1] = ps_mm.tile([128, 512], F32, tag="mm", name=f"psyB{p}")
            if tk >= 5 and (tk - 5) % 2 == 0 and nextq < NPAIR:
                emit_attnv_pair(p, nextq, ex.pop(nextq), ps_y2)
                nextq += 1
            if kwork and p > 0 and tk >= 4:
                o2, w2, ci, t = kwork.pop(0)
                emit_k_chunk_mm(kch[p + 1], p + 1, o2, w2, ci, t)
        while kwork:
            o2, w2, ci, t = kwork.pop(0)
            emit_k_chunk_mm(kch[p + 1], p + 1, o2, w2, ci, t)
        while nextq < NPAIR:
            emit_attnv_pair(p, nextq, ex.pop(nextq), ps_y2)
            nextq += 1
        pend_r.append(dict(p=p, ps_y2=ps_y2))
        if "k" in dbg:
            for (o2, w2) in KT:
                dk_ = P4.tile([128, 512], F32, tag="dbgt", name=f"dbk{p}_{o2}", bufs=1)
                nc.vector.tensor_copy(out=dk_[:, 0:w2], in_=kch[p][:, o2:o2 + w2])
                nc.sync.dma_start(out=dbg["k"][ts(p, 128), o2:o2 + w2],
                                  in_=dk_[:, 0:w2])
    pr_ = pend_r.pop()
    r65s = emit_recips(pr_["p"], pr_["ps_y2"])
    emit_deferred_epilogue(p=pr_["p"], ps_y2=pr_["ps_y2"], r65s=r65s)
    proj_pre = {}
    ps_us = ps_mm.tile([1, NQ], F32, tag="mm", name="ps_us")
    for c in range(C6):
        nc.tensor.matmul(ps_us, uproj_s[:, c:c + 1], y_sb[:, c, :],
                         start=(c == 0), stop=(c == C6 - 1))
    if "y" in dbg:
        for c in range(C6):
            dy_ = P4.tile([128, 512], F32, tag="dbgt", name=f"dby{c}", bufs=1)
            nc.vector.tensor_copy(out=dy_, in_=y_sb[:, c, :])
            nc.sync.dma_start(out=dbg["y"][ts(c, 128), :], in_=dy_)

    # ========== proj (fp8 DR, PE-centered) -> d2 = x2 - mean2 ==========
    d2_sb = P1.tile([128, C6, NQ], F32, tag="x2")
    # LN2 mean rows (need only u.y + sumx0)
    srow = P6.tile([1, NQ], F32, tag="rowf", bufs=4, name="srow2")
    nc.vector.scalar_tensor_tensor(out=srow, in0=ps_us, scalar=float(sbp),
                                   in1=sumx0, op0=OP.add, op1=OP.add)
    mrow2n = P6.tile([1, NQ], BF16, tag="row", bufs=6, name="mrow2n")
    nc.vector.tensor_scalar(out=mrow2n, in0=srow, scalar1=-WS / D,
                            scalar2=None, op0=OP.mult)
    m2row = P6.tile([1, NQ], BF16, tag="row", bufs=6, name="m2row")
    nc.vector.tensor_scalar(out=m2row, in0=srow, scalar1=1.0 / D,
                            scalar2=None, op0=OP.mult)

    sq2 = P4.tile([128, C6, NQ], FP8, tag="sq", name="sq2", bufs=2)
    for co in range(C6):
        if co in proj_pre:
            ps = proj_pre[co]
            nc.tensor.matmul(ps, wproj8[:, 4:6, ts(co, 128)],
                             y_sb[:, 4:6, :],
                             start=False, stop=False, perf_mode=DR)
        else:
            ps = ps_mm.tile([128, NQ], F32, tag="mm")
            for ci in range(0, C6, 2):
                nc.tensor.matmul(ps, wproj8[:, ci:ci + 2, ts(co, 128)],
                                 y_sb[:, ci:ci + 2, :],
                                 start=(ci == 0), stop=False, perf_mode=DR)
        # center in-psum: ps += 1 (x) (-WS m2)
        nc.tensor.matmul(ps, onesb[0:1, :], mrow2n, start=False, stop=True)
        nc.vector.scalar_tensor_tensor(out=d2_sb[:, co, :], in0=ps,
                                       scalar=1.0 / WS,
                                       in1=xb_sb[:, co, :],
                                       op0=OP.mult, op1=OP.add)
        nc.scalar.activation(sq2[:, co, :], d2_sb[:, co, :], AF.Square)

    if "x2" in dbg:
        for c in range(C6):
            dx2_ = P4.tile([128, 512], F32, tag="dbgt", name=f"dbx2{c}", bufs=1)
            nc.vector.tensor_copy(out=dx2_, in_=d2_sb[:, c, :])
            nc.sync.dma_start(out=dbg["x2"][ts(c, 128), :], in_=dx2_)

    # LN2 var + rstd (var = E[d2^2] exactly -- d2 already centered)
    ps_sq2 = ps_mm.tile([1, NQ], F32, tag="mm", name="ps_sq2")
    for ci in range(0, C6, 2):
        nc.tensor.matmul(ps_sq2, ones8, sq2[:, ci:ci + 2, :],
                         start=(ci == 0), stop=(ci == C6 - 2), perf_mode=DR)
    vrow2 = P6.tile([1, NQ], F32, tag="rowf", bufs=4, name="vrow2")
    nc.vector.tensor_scalar(out=vrow2, in0=ps_sq2, scalar1=1.0 / D,
                            scalar2=None, op0=OP.mult)
    srt2 = P6.tile([1, NQ], F32, tag="rowf", bufs=4, name="srt2")
    nc.scalar.activation(srt2, vrow2, AF.Sqrt, bias=eps1)
    # warm the gelu table while PE runs proj/fc1 lead-in
    warmg = P6.tile([1, 8], F32, tag="rowf", bufs=4, name="warmg")
    nc.scalar.activation(warmg, srt2[0:1, 0:8], AF.Gelu)
    rf2 = P6.tile([1, NQ], F32, tag="rowf", bufs=4, name="rf2")
    nc.vector.reciprocal_approx_fast(out=rf2, in_=srt2)
    rrow2 = P6.tile([1, NQ], BF16, tag="row", bufs=6, name="rrow2")
    nc.vector.tensor_copy(out=rrow2, in_=rf2)
    prb2 = ps_mm.tile([128, NQ], F32, tag="mm", name="prb2")
    nc.tensor.matmul(prb2, onesb[0:1, :], rrow2, start=True, stop=True)
    xn2 = P2.tile([128, C6, NQ], FP8, tag="xn2", bufs=1)
    for c in range(C6):
        nc.vector.tensor_tensor(xn2[:, c, :], d2_sb[:, c, :], prb2,
                                op=OP.mult)

    if "xn2" in dbg:
        for c in range(C6):
            dn_ = P4.tile([128, 512], F32, tag="dbgt", name=f"dbn{c}", bufs=1)
            nc.vector.tensor_copy(out=dn_, in_=xn2[:, c, :])
            nc.sync.dma_start(out=dbg["xn2"][ts(c, 128), :], in_=dn_)

    # ========== MLP: fc1 fp8 DR -> gelu; fc2 bf16 ==========
    h_sb = P2.tile([128, HO24, NQ], BF16, tag="h", bufs=1)
    wfc2b = None
    for ho in range(HO24):
        if ho == 12:
            wfc2b = P2.tile([128, 12, D], BF16, tag="wbig", bufs=2)
            nc.sync.dma_start(
                out=wfc2b,
                in_=wfc2_d.rearrange("(a p) o -> p a o", p=128)[:, 12:24, :])
        ps = ps_mm.tile([128, NQ], F32, tag="mm")
        for ci in range(0, C6, 2):
            nc.tensor.matmul(ps, wfc1_sb[:, ci:ci + 2, ts(ho, 128)],
                             xn2[:, ci:ci + 2, :],
                             start=(ci == 0), stop=(ci == C6 - 2), perf_mode=DR)
        nc.scalar.activation(h_sb[:, ho, :], ps, AF.Gelu,
                             bias=bfc1_s[:, ho:ho + 1], scale=1.0 / WS)
    for co in range(C6):
        ps = ps_mm.tile([128, NQ], F32, tag="mm")
        for ho in range(HO24):
            wsrc = wfc2a if ho < 12 else wfc2b
            nc.tensor.matmul(ps, wsrc[:, ho % 12, ts(co, 128)], h_sb[:, ho, :],
                             start=(ho == 0), stop=False)
        # de-center the residual: ps += 1 (x) m2 (out = d2 + m2 + mlp)
        nc.tensor.matmul(ps, onesb[0:1, :], m2row, start=False, stop=True)
        o = P2.tile([128, NQ], F32, tag="ot")
        nc.vector.scalar_tensor_tensor(out=o, in0=ps,
                                       scalar=bfc2_s[:, co:co + 1],
                                       in1=d2_sb[:, co, :],
                                       op0=OP.add, op1=OP.add)
        nc.sync.dma_start(out=out_d[ts(co, 128), :], in_=o)

    for cm in (ps_sc_cm, ps_mm_cm, P6_cm, P4_cm, P2_cm, P1_cm):
        cm.__exit__(None, None, None)


def _host_prep(x, mask, ln1_g, ln1_b, qkv_w, proj_w, proj_b, ln2_g, ln2_b,
               fc1_w, fc1_b, fc2_w, fc2_b):
    bf = ml_dtypes.bfloat16
    fp8 = ml_dtypes.float8_e4m3
    f32 = np.float32
    x = np.asarray(x, f32)
    mask = np.asarray(mask)
    qkv_w = np.asarray(qkv_w, f32)
    proj_w = np.asarray(proj_w, f32)
    fc1_w = np.asarray(fc1_w, f32)
    fc2_w = np.asarray(fc2_w, f32)
    ln1_g = np.asarray(ln1_g, f32); ln1_b = np.asarray(ln1_b, f32)
    ln2_g = np.asarray(ln2_g, f32); ln2_b = np.asarray(ln2_b, f32)
    proj_b = np.asarray(proj_b, f32)
    fc1_b = np.asarray(fc1_b, f32); fc2_b = np.asarray(fc2_b, f32)

    # the raw-G scores fold (rstd on the exp scale) needs zero qkv bias,
    # which holds iff ln1_b == 0 (the reference qkv matmul has no bias)
    assert np.all(ln1_b == 0.0), "kernel assumes ln1_b == 0"

    wqkv_f = qkv_w * ln1_g[None, :]
    wqkv_f[0:D] *= SCALE
    bproj_f = proj_b
    cs = wqkv_f.sum(axis=1)          # [2304] per-output-channel colsums
    wfc1_f = fc1_w * ln2_g[None, :]
    bfc1_f = fc1_w @ ln2_b + fc1_b

    shared = {
        "wqkv": np.ascontiguousarray(wqkv_f.T * WS).astype(fp8),
        "wproj": np.ascontiguousarray(proj_w.T * WS).astype(fp8),
        "wfc1": np.ascontiguousarray(wfc1_f.T * WS).astype(fp8),
        "wfc2": np.ascontiguousarray(fc2_w.T).astype(bf),
        "uproj": np.ascontiguousarray(
            proj_w.sum(axis=0).reshape(6, 128).T).astype(bf),
        "csrows": np.ascontiguousarray((-WS * cs)[None, :]).astype(bf),
    }
    sbp = float(bproj_f.sum())

    # compacted unmasked keys per batch
    kidx = [np.where(mask[b] != 1)[0] for b in range(B)]
    nk_max = max(len(i) for i in kidx)
    NK = max(((nk_max + 255) // 256) * 256, 512)

    # packed f32 constants: m01 (per-core) | csq | csk | bfc1 | bfc2
    constf_tail = np.concatenate([
        (-WS * cs[0:D]).reshape(6, 128).T,
        (-WS * cs[D:2 * D]).reshape(6, 128).T,
        bfc1_f.reshape(24, 128).T,
        fc2_b.reshape(6, 128).T,
    ], axis=1).astype(f32)

    in_maps = []
    for core in range(NC):
        b, s = divmod(core, NSH)
        xq = x[b][s * NQ:(s + 1) * NQ]       # [NQ, D]
        nk = len(kidx[b])
        xk = np.zeros((NK, D), f32)
        xk[:nk] = x[b][kidx[b]]
        m01k = np.zeros((NK,), f32)
        m01k[:nk] = 1.0
        im = dict(shared)
        im["xq8"] = np.ascontiguousarray(xq.T).astype(fp8)
        im["xk8"] = np.ascontiguousarray(xk.T).astype(fp8)
        im["xb"] = np.ascontiguousarray(xq.T + bproj_f[:, None]).astype(bf)
        im["constf"] = np.ascontiguousarray(np.concatenate(
            [m01k.reshape(NK // 128, 128).T.astype(f32), constf_tail],
            axis=1))
        in_maps.append(im)
    return in_maps, sbp, NK


def kernel(**inputs):
    in_maps, sbp, NK = _host_prep(**inputs)
    if _cached.get("key") != (sbp, NK):
        _cached["nc"] = _build_nc(sbp, NK)
        _cached["key"] = (sbp, NK)
    res = run_bass_kernel_spmd(_cached["nc"], in_maps, core_ids=list(range(NC)))
    out = np.empty((B, N, D), np.float32)
    for core in range(NC):
        b, s = divmod(core, NSH)
        out[b, s * NQ:(s + 1) * NQ, :] = res.results[core]["out"].T
    return out


# revision 32
# speedup vs baseline: 1.0235x; 1.0115x over previous
"""Trainium2 Bass kernel for a dense transformer block (pre-LN, MHA + GELU MLP).

Problem shapes (hardcoded): x [2, 2048, 768] f32, mask [2, 2048] int32,
12 heads x 64 dims, hidden 3072.

Sharding: 8 cores = (batch b in {0,1}) x (token shard s in {0..3}).
Each core handles its 512-query shard of Q / attention rows / MLP / output.

Key compaction: the key-padding mask kills ~half the keys, so the host
gathers only the unmasked keys' x columns into xk [D, NK] (NK = count
rounded up to 256, zero-padded). Pad keys are neutralized by zeroing their
V rows and ones-column entries.

LN1 is never applied to activations ("raw-G" trick): Q/K/V matmuls run on
raw fp8 x directly (so the PE starts as soon as DMA lands, with no
LN-apply serialization), and the normalization is folded into the
epilogues via per-output-channel weight column sums:
    q[o,t] = r[t] * (G[o,t] - m[t]*colsum_w[o])        (valid: ln1_b == 0)
The query-side rstd folds into the Q epilogue; the key-side rstd folds
into the exp's per-partition ACT scale (keys sit on PSUM partitions for
scores), and the V-side mean/rstd fold into the V epilogue using
transposed per-key mean/rstd columns (tiny PE transpose matmuls).

On-chip layout is feature-major: activations are [features, tokens],
every matmul contracts over the partition dim. Per-token LN stats come
from ones-vector DoubleRow matmuls on raw fp8 x; the fp8 squares are
split across DVE/ACT/GpSimd so no one engine serializes the startup.
Softmax denominators come free from a ones column appended to V (M=65
matmul). Max-subtraction is skipped: |scores| <= ~4 by construction.
LN2 mean is folded into the proj PSUM as a K=1 rank-1 accumulate (and
un-folded in the fc2 PSUM), so the residual stream lives centered (d2)
and the LN2 apply is one multiply.

fp8 DoubleRow everywhere except fc2 (kept bf16: quantizing the 3072-wide
contraction would eat too much of the 2e-2 error budget). DMAs are one
issue per tensor (descriptor issue costs ~600ns on the Sync engine) in
consumer-priority order. Perf note for this box: the PE runs at 1.2 GHz
through dependency-mixed regions (only long uniform MM streams reach
2.4 GHz), so attention-path epilogues belong on DVE, not on the PE.
"""

import numpy as np
import ml_dtypes

import concourse.bass as bass
import concourse.tile as tile
import concourse.mybir as mybir
from concourse import bacc
from concourse.bass import ts
from concourse.bass_utils import run_bass_kernel_spmd
from concourse.alu_op_type import AluOpType

BF16 = mybir.dt.bfloat16
F32 = mybir.dt.float32
FP8 = mybir.dt.float8e4
DR = mybir.MatmulPerfMode.DoubleRow
WS = 32.0   # fp8 weight scale (dodges e4m3 subnormals)

B = 2
N = 2048
D = 768
H = 12
HD = 64
HID = 3072
EPS = 1e-5
SCALE = HD ** -0.5
NQ = 512          # queries per core
NSH = N // NQ     # token shards per batch
NC = B * NSH      # 8 cores
C6 = D // 128     # feature chunks
HO24 = HID // 128

AF = mybir.ActivationFunctionType
OP = AluOpType

_cached = {}
_rid = [0]


def _rid_next():
    _rid[0] += 1
    return _rid[0]


def _build_nc(sbp, NK):
    nc = bacc.Bacc("TRN2", target_bir_lowering=False, debug=False,
                   enable_asserts=False, num_devices=NC)

    xq8 = nc.dram_tensor("xq8", [D, NQ], FP8, kind="ExternalInput").ap()
    xk8 = nc.dram_tensor("xk8", [D, NK], FP8, kind="ExternalInput").ap()
    xb = nc.dram_tensor("xb", [D, NQ], BF16, kind="ExternalInput").ap()
    wqkv = nc.dram_tensor("wqkv", [D, 3 * D], FP8, kind="ExternalInput").ap()
    wproj = nc.dram_tensor("wproj", [D, D], FP8, kind="ExternalInput").ap()
    wfc1 = nc.dram_tensor("wfc1", [D, HID], FP8, kind="ExternalInput").ap()
    wfc2 = nc.dram_tensor("wfc2", [HID, D], BF16, kind="ExternalInput").ap()
    uproj = nc.dram_tensor("uproj", [128, 6], BF16, kind="ExternalInput").ap()
    # negated WS-scaled weight column sums as rows: csq | csk | csv
    csrows = nc.dram_tensor("csrows", [1, 3 * D], BF16,
                            kind="ExternalInput").ap()
    KC = NK // 128
    # packed f32 constants: m01 | csq | csk | bfc1 | bfc2
    constf = nc.dram_tensor("constf", [128, KC + 42], F32,
                            kind="ExternalInput").ap()
    out_d = nc.dram_tensor("out", [D, NQ], F32, kind="ExternalOutput").ap()
    import os
    dbg = {}
    if os.environ.get("KDBG"):
        dbg["y"] = nc.dram_tensor("dbg_y", [D, NQ], F32, kind="ExternalOutput").ap()
        dbg["x2"] = nc.dram_tensor("dbg_x2", [D, NQ], F32, kind="ExternalOutput").ap()
        dbg["q"] = nc.dram_tensor("dbg_q", [D, NQ], F32, kind="ExternalOutput").ap()
        dbg["k"] = nc.dram_tensor("dbg_k", [D, NK], F32, kind="ExternalOutput").ap()
        dbg["xn2"] = nc.dram_tensor("dbg_xn2", [D, NQ], F32, kind="ExternalOutput").ap()

    with tile.TileContext(nc) as tc:
        _body(nc, tc, sbp, NK, xq8, xk8, xb, wqkv, wproj, wfc1, wfc2,
              uproj, csrows, constf, out_d, dbg)
    nc.compile()
    return nc


def _body(nc, tc, sbp, NK, xq8_d, xk8_d, xb_d, wqkv_d, wproj_d, wfc1_d,
          wfc2_d, uproj_d, csrows_d, constf_d, out_d, dbg=None):
    dbg = dbg or {}
    KC = NK // 128
    NPAIR = KC // 2
    KT = []
    off = 0
    while off < NK:
        w = min(512, NK - off)
        KT.append((off, w))
        off += w
    NT = len(KT)

    P1_cm = tc.tile_pool(name="p1", bufs=1); P1 = P1_cm.__enter__()
    P2_cm = tc.tile_pool(name="p2", bufs=2); P2 = P2_cm.__enter__()
    P4_cm = tc.tile_pool(name="p4", bufs=4); P4 = P4_cm.__enter__()
    P6_cm = tc.tile_pool(name="p6", bufs=6); P6 = P6_cm.__enter__()
    ps_mm_cm = tc.tile_pool(name="ps_mm", bufs=4, space="PSUM")
    ps_mm = ps_mm_cm.__enter__()
    ps_sc_cm = tc.tile_pool(name="ps_sc", bufs=2, space="PSUM")
    ps_sc = ps_sc_cm.__enter__()

    # ---- DMAs: one issue per tensor (issue cost ~600ns each on Sync),
    # ordered by when the consumer needs the data ----
    xq8_r = xq8_d.rearrange("(a p) t -> p a t", p=128)
    xk8_r = xk8_d.rearrange("(a p) t -> p a t", p=128)
    wqkv_r = wqkv_d.rearrange("(a p) o -> p a o", p=128)
    x8 = P1.tile([128, C6, NQ], FP8, tag="x8")
    nc.sync.dma_start(out=x8[:, 0:2, :], in_=xq8_r[:, 0:2, :])
    nc.sync.dma_start(out=x8[:, 2:C6, :], in_=xq8_r[:, 2:C6, :])
    w_sb = P2.tile([128, C6, 3 * D], FP8, tag="wbig", bufs=2)
    nc.sync.dma_start(out=w_sb[:, :, 0:D], in_=wqkv_r[:, :, 0:D])
    xk8 = P1.tile([128, C6, NK], FP8, tag="xk8")
    for (o2, w2) in KT:
        nc.sync.dma_start(out=xk8[:, :, o2:o2 + w2], in_=xk8_r[:, :, o2:o2 + w2])
    # packed f32 constants: m01 | csq | csk | bfc1 | bfc2
    constf = P1.tile([128, KC + 42], F32, tag="constf")
    nc.sync.dma_start(out=constf, in_=constf_d)
    m01 = constf[:, 0:KC]
    csq_s = constf[:, KC:KC + 6]
    csk_s = constf[:, KC + 6:KC + 12]
    bfc1_s = constf[:, KC + 12:KC + 36]
    bfc2_s = constf[:, KC + 36:KC + 42]
    nc.sync.dma_start(out=w_sb[:, :, D:2 * D], in_=wqkv_r[:, :, D:2 * D])
    csr = P1.tile([1, 3 * D], BF16, tag="csr")
    nc.sync.dma_start(out=csr, in_=csrows_d)
    csq_row = csr[:, 0:D]
    csk_row = csr[:, D:2 * D]
    csv_row = csr[:, 2 * D:3 * D]
    nc.sync.dma_start(out=w_sb[:, :, 2 * D:3 * D], in_=wqkv_r[:, :, 2 * D:3 * D])
    uproj_s = P1.tile([128, 6], BF16, tag="uproj")
    nc.sync.dma_start(out=uproj_s, in_=uproj_d)

    # ---- on-chip constants ----
    onesb = P1.tile([128, 128], BF16, tag="onesb")
    nc.vector.memset(onesb, 1.0)
    ones8_t = P1.tile([128, 2, 16], FP8, tag="ones8")
    nc.vector.memset(ones8_t, 1.0)
    ones8 = ones8_t[:, :, 0:1]
    eps1 = P1.tile([1, 1], F32, tag="eps1")
    nc.vector.memset(eps1, EPS)
    sumx0 = P1.tile([1, NQ], BF16, tag="sumx0")
    # V+denominator store; ones/mask column per head seeded early (the
    # strided fp8 scatter costs ~2us on DVE -- keep it off the row-chain path)
    vsb = P1.tile([128, KC, 16 * ((H * (HD + 1) + 15) // 16)], FP8, tag="vsb")
    m01r = bass.AP(tensor=m01.tensor, offset=m01.offset,
                   ap=[list(m01.ap[0]), list(m01.ap[1]), [0, H], [0, 1]])
    vsb_h = vsb[:, :, 0:H * (HD + 1)].rearrange("p k (h e) -> p k h e",
                                                e=HD + 1)
    nc.vector.tensor_copy(out=vsb_h[:, :, :, HD:HD + 1], in_=m01r)

    # ---- later-phase DMAs (behind the attention-critical ones) ----
    xb_sb = P1.tile([128, C6, NQ], BF16, tag="xb")
    nc.sync.dma_start(out=xb_sb, in_=xb_d.rearrange("(a p) t -> p a t", p=128))
    wproj8 = P1.tile([128, C6, D], FP8, tag="wpj")
    nc.sync.dma_start(out=wproj8, in_=wproj_d.rearrange("(a p) o -> p a o", p=128))
    wfc1_sb = P1.tile([128, C6, HID], FP8, tag="wfc1")
    nc.sync.dma_start(out=wfc1_sb, in_=wfc1_d.rearrange("(a p) o -> p a o", p=128))
    wfc2a = P2.tile([128, 12, D], BF16, tag="wbig", bufs=2)
    nc.sync.dma_start(out=wfc2a,
                      in_=wfc2_d.rearrange("(a p) o -> p a o", p=128)[:, 0:12, :])

    # ========== LN stats on raw fp8 x (sum/sumsq via DR matmuls) ==========
    def emit_stats(src, t_off, w, nm, sqeng="dve"):
        i = _rid_next()
        ps_sum = ps_mm.tile([1, w], F32, tag="mm", name=f"pssum{i}")
        ps_sq = ps_mm.tile([1, w], F32, tag="mm", name=f"pssq{i}")
        for ci in range(0, C6, 2):
            nc.tensor.matmul(ps_sum, ones8, src[:, ci:ci + 2, t_off:t_off + w],
                             start=(ci == 0), stop=(ci == C6 - 2), perf_mode=DR)
        sq = P4.tile([128, C6, w], FP8, tag="sq", name=f"sq{i}", bufs=2)
        for ci in range(0, C6, 2):
            if sqeng == "act":
                nc.scalar.activation(sq[:, ci:ci + 2, :],
                                     src[:, ci:ci + 2, t_off:t_off + w],
                                     AF.Square)
            elif sqeng == "gps":
                nc.gpsimd.tensor_tensor(sq[:, ci:ci + 2, :],
                                        src[:, ci:ci + 2, t_off:t_off + w],
                                        src[:, ci:ci + 2, t_off:t_off + w],
                                        op=OP.mult)
            else:
                nc.vector.tensor_tensor(sq[:, ci:ci + 2, :],
                                        src[:, ci:ci + 2, t_off:t_off + w],
                                        src[:, ci:ci + 2, t_off:t_off + w],
                                        op=OP.mult)
        for ci in range(0, C6, 2):
            nc.tensor.matmul(ps_sq, ones8, sq[:, ci:ci + 2, :],
                             start=(ci == 0), stop=(ci == C6 - 2), perf_mode=DR)
        mrow = P6.tile([1, w], BF16, tag="row", bufs=6, name=f"mrow{i}")
        nc.vector.tensor_scalar(out=mrow, in0=ps_sum, scalar1=1.0 / D,
                                scalar2=None, op0=OP.mult)
        m2 = P6.tile([1, w], F32, tag="rowf", bufs=4, name=f"m2_{i}")
        nc.vector.tensor_tensor(m2, mrow, mrow, op=OP.mult)
        vrow = P6.tile([1, w], F32, tag="rowf", bufs=4, name=f"vrow{i}")
        nc.vector.scalar_tensor_tensor(out=vrow, in0=ps_sq, scalar=1.0 / D,
                                       in1=m2, op0=OP.mult, op1=OP.subtract)
        srt = P6.tile([1, w], F32, tag="rowf", bufs=4, name=f"srt{i}")
        nc.scalar.activation(srt, vrow, AF.Sqrt, bias=eps1)
        rf = P6.tile([1, w], F32, tag="rowf", bufs=4, name=f"rf{i}")
        nc.vector.reciprocal_approx_fast(out=rf, in_=srt)
        rrow = P6.tile([1, w], BF16, tag="row", bufs=6, name=f"rrow{i}")
        nc.vector.tensor_copy(out=rrow, in_=rf)
        return ps_sum, mrow, rrow

    # ---- broadcast tiles (PE K=1 matmul + copy) ----
    def emit_bcast(row, w, nm):
        pmb = ps_mm.tile([128, w], F32, tag="mm", name=f"pmb_{nm}")
        nc.tensor.matmul(pmb, onesb[0:1, :], row, start=True, stop=True)
        t = P4.tile([128, w], BF16, tag="bcs", name=f"bc_{nm}", bufs=4)
        nc.vector.tensor_copy(out=t, in_=pmb)
        return t

    # ---- per-key mean/rstd columns via tiny PE transposes ----
    mkc = P1.tile([128, KC], F32, tag="mkc")
    expsc = P1.tile([128, KC], F32, tag="expsc")
    vscale = P1.tile([128, KC], F32, tag="vscale")
    avcol = P1.tile([128, KC], F32, tag="avcol")

    def emit_kcols(t):
        o2, w2 = KT[t]
        nch = w2 // 128
        c0 = o2 // 128
        i = _rid_next()
        pm = ps_mm.tile([128, nch], F32, tag="mm", name=f"pstm{i}")
        pr = ps_mm.tile([128, nch], F32, tag="mm", name=f"pstr{i}")
        for k in range(nch):
            nc.tensor.matmul(pm[:, k:k + 1],
                             kstats[t][1][0:1, k * 128:(k + 1) * 128],
                             onesb[0:1, 0:1], start=True, stop=True)
            nc.tensor.matmul(pr[:, k:k + 1],
                             kstats[t][2][0:1, k * 128:(k + 1) * 128],
                             onesb[0:1, 0:1], start=True, stop=True)
        nc.vector.tensor_copy(out=mkc[:, c0:c0 + nch], in_=pm)
        nc.vector.tensor_scalar(out=expsc[:, c0:c0 + nch], in0=pr,
                                scalar1=1.0 / WS, scalar2=None, op0=OP.mult)
        nc.vector.tensor_tensor(vscale[:, c0:c0 + nch], expsc[:, c0:c0 + nch],
                                m01[:, c0:c0 + nch], op=OP.mult)
        nc.vector.tensor_tensor(avcol[:, c0:c0 + nch], mkc[:, c0:c0 + nch],
                                vscale[:, c0:c0 + nch], op=OP.mult)

    # stats: queries, then key tile 0 (tile 1 deferred until after the
    # first-scores dependencies so exp can start ASAP)
    ps_sum_q, mrow_q, rrow_q = emit_stats(x8, 0, NQ, "q")
    nc.vector.tensor_copy(out=sumx0, in_=ps_sum_q)
    kstats = [None] * NT
    mb_k = [None] * NT
    for t in range(NT):
        kstats[t] = emit_stats(xk8, KT[t][0], KT[t][1], f"k{t}", sqeng="act")
        mb_k[t] = emit_bcast(kstats[t][1], KT[t][1], f"mk{t}")
        emit_kcols(t)
    mb_q = emit_bcast(mrow_q, NQ, "mq")
    rbrow = P6.tile([1, NQ], BF16, tag="row", bufs=6, name="rbrow")
    nc.vector.tensor_scalar(out=rbrow, in0=rrow_q, scalar1=1.0 / WS,
                            scalar2=None, op0=OP.mult)
    rb_q = emit_bcast(rbrow, NQ, "rq")

    # ========== Q (raw-G + folded epilogue) ==========
    qT = P2.tile([128, C6, NQ], BF16, tag="qT", bufs=1)
    for co in range(C6):
        ps = ps_mm.tile([128, NQ], F32, tag="mm")
        for ci in range(0, C6, 2):
            nc.tensor.matmul(ps, w_sb[:, ci:ci + 2, ts(co, 128)],
                             x8[:, ci:ci + 2, :],
                             start=(ci == 0), stop=(ci == C6 - 2), perf_mode=DR)
        t1 = P4.tile([128, NQ], BF16, tag="tmp", name=f"qt1_{co}")
        nc.vector.scalar_tensor_tensor(out=t1, in0=mb_q,
                                       scalar=csq_s[:, co:co + 1], in1=ps,
                                       op0=OP.mult, op1=OP.add)
        nc.vector.tensor_tensor(qT[:, co, :], t1, rb_q, op=OP.mult)

    if "q" in dbg:
        for c in range(C6):
            dq_ = P4.tile([128, 512], F32, tag="dbgt", name=f"dbq{c}", bufs=1)
            nc.vector.tensor_copy(out=dq_, in_=qT[:, c, :])
            nc.sync.dma_start(out=dbg["q"][ts(c, 128), :], in_=dq_)

    # ========== attention pipeline ==========

    kch_state = [None]

    def emit_k_chunk_mm(kch_p, p, o2, w2, ci, t):
        if ci == 0:
            kst = ps_mm.tile([128, w2], F32, tag="mm", name=f"kst{_rid_next()}")
            kch_state[0] = kst
        nc.tensor.matmul(kch_state[0], w_sb[:, ci:ci + 2, ts(6 + p, 128)],
                         xk8[:, ci:ci + 2, o2:o2 + w2],
                         start=(ci == 0), stop=(ci == C6 - 2), perf_mode=DR)
        if ci == C6 - 2:
            nc.vector.scalar_tensor_tensor(out=kch_p[:, o2:o2 + w2],
                                           in0=mb_k[t],
                                           scalar=csk_s[:, p:p + 1],
                                           in1=kch_state[0],
                                           op0=OP.mult, op1=OP.add)
            kch_state[0] = None

    def emit_v_chunk(tk):
        i = _rid_next()
        for half in range(2):
            psv = ps_mm.tile([128, 512], F32, tag="mm", name=f"psv{tk}_{half}")
            for ci in range(0, C6, 2):
                nc.tensor.matmul(psv[:, 0:384],
                                 xk8[:, ci:ci + 2, ts(tk, 128)],
                                 w_sb[:, ci:ci + 2, 12 * 128 + half * 384:
                                      12 * 128 + (half + 1) * 384],
                                 start=(ci == 0), stop=(ci == C6 - 2),
                                 perf_mode=DR)
            t1 = P4.tile([128, 384], BF16, tag="vt", name=f"vt{i}_{half}",
                         bufs=4)
            nc.vector.tensor_scalar(out=t1, in0=psv[:, 0:384],
                                    scalar1=vscale[:, tk:tk + 1],
                                    scalar2=None, op0=OP.mult)
            vout = vsb[:, tk, half * 390:half * 390 + 390].rearrange(
                "p (h e) -> p h e", e=HD + 1)[:, :, 0:HD]
            vin = t1.rearrange("p (h d) -> p h d", h=6)
            csvv = csv_t[:, half, :].rearrange("p (h d) -> p h d", h=6)
            nc.vector.scalar_tensor_tensor(out=vout, in0=csvv,
                                           scalar=avcol[:, tk:tk + 1],
                                           in1=vin, op0=OP.mult, op1=OP.add)

    def emit_attnv_pair(p, q, ex2t, ps_y2):
        for j in range(2):
            h = 2 * p + j
            nc.tensor.matmul(ps_y2[j][0:HD + 1, :],
                             vsb[:, 2 * q:2 * q + 2, h * 65:h * 65 + 65],
                             ex2t[:, :, j, :],
                             start=(q == 0), stop=(q == NPAIR - 1),
                             perf_mode=DR)

    def emit_recips(p, ps_y2, pool=None, ptag="mm"):
        pool = pool or ps_mm
        r65s = []
        for j in range(2):
            sr = P4.tile([128, 512], BF16, tag="tf", name=f"sr{p}_{j}")
            nc.vector.tensor_copy(out=sr[HD:HD + 1, :],
                                  in_=ps_y2[j][HD:HD + 1, :])
            psb = pool.tile([64, 512], F32, tag=ptag, name=f"psb{p}_{j}")
            nc.tensor.matmul(psb, onesb[HD:HD + 1, 0:HD], sr[HD:HD + 1, :],
                             start=True, stop=True)
            rbf = P4.tile([128, 512], F32, tag="tf", name=f"rbf{p}_{j}")
            nc.vector.reciprocal_approx_fast(out=rbf[0:HD, :], in_=psb[0:HD, :])
            r65s.append(rbf)
        return r65s

    def emit_deferred_epilogue(p, ps_y2, r65s):
        for j in range(2):
            ps_y = ps_y2[j]
            if j == 0:
                nc.vector.tensor_tensor(y_sb[0:HD, p, :], ps_y[0:HD, :],
                                        r65s[j][0:HD, :], op=OP.mult)
            else:
                yt = P4.tile([128, 512], FP8, tag="yt", name=f"yt{p}")
                nc.vector.tensor_tensor(yt[0:HD, :], ps_y[0:HD, :],
                                        r65s[j][0:HD, :], op=OP.mult)
                nc.sync.dma_start(out=y_sb[HD:128, p, :], in_=yt[0:HD, :])

    y_sb = P1.tile([128, C6, NQ], FP8, tag="y")
    kch = {}
    pend = []
    pend_r = []

    # K(0) tile 0 first (unblocks scores/exp), then deferred key-tile-1
    # stats, then K(0) tile 1; V chunks go inside the p=0 loop
    kch[0] = P2.tile([128, NK], BF16, tag="kch", name="kch0")
    for t in range(NT):
        for ci in range(0, C6, 2):
            emit_k_chunk_mm(kch[0], 0, KT[t][0], KT[t][1], ci, t)
    # csv broadcast tile [128, 2, 384] (negated, x WS host-side)
    csv_t = P1.tile([128, 2, 384], BF16, tag="csvt")
    for half in range(2):
        pmcv = ps_mm.tile([128, 384], F32, tag="mm", name=f"pmcsv{half}")
        nc.tensor.matmul(pmcv, onesb[0:1, :],
                         csv_row[0:1, half * 384:(half + 1) * 384],
                         start=True, stop=True)
        nc.vector.tensor_copy(out=csv_t[:, half, :], in_=pmcv)
    # warm the exp ACT table (after the LN1 Sqrt set, before first exp)
    warme = P1.tile([1, 8], F32, tag="warme")
    nc.scalar.activation(warme, onesb[0:1, 0:8], AF.Exp)
    emit_v_chunk(0)
    emit_v_chunk(1)
    emit_v_chunk(2)
    emit_v_chunk(3)

    for p in range(C6):
        if p < C6 - 1:
            kch[p + 1] = P2.tile([128, NK], BF16, tag="kch", name=f"kch{p + 1}")
            kwork = [(o2, w2, ci, t) for t, (o2, w2) in enumerate(KT)
                     for ci in range(0, C6, 2)]
        else:
            kwork = []
        ex = {}
        ps_y2 = [None, None]
        nextq = 0
        for tk in range(KC):
            pss = ps_sc.tile([128, 2, 512], F32, tag="sc")
            for j in range(2):
                po = j * 64
                nc.tensor.matmul(pss[:, j, :],
                                 kch[p][po:po + 64, ts(tk, 128)],
                                 qT[po:po + 64, p, 0:NQ],
                                 start=True, stop=True)
            if tk % 2 == 0:
                ex[tk // 2] = P6.tile([128, 2, 2, 512], FP8, tag="exp",
                                      name=f"ex_{p}_{tk // 2}", bufs=4)
            nc.scalar.activation(ex[tk // 2][:, tk % 2, :, :], pss, AF.Exp,
                                 scale=expsc[:, tk:tk + 1])
            if p == 0 and 0 <= tk <= KC - 5:
                emit_v_chunk(tk + 4)
            # recips/epilogue of the PREVIOUS p are emitted here, behind this
            # p's first scores, so the PE never stalls on the DVE row chain
            # at the p-boundary (the stall tripped the clock-gate every p)
            if tk == 1 and pend_r:
                pr_ = pend_r.pop()
                r65s = emit_recips(pr_["p"], pr_["ps_y2"])
                pend.append(dict(p=pr_["p"], ps_y2=pr_["ps_y2"], r65s=r65s))
            if tk == 3 and pend:
                emit_deferred_epilogue(**pend.pop())
            if tk == 5:
                ps_y2[0] = ps_mm.tile([128, 512], F32, tag="mm", name=f"psyA{p}")
                ps_y2[1] = ps_mm.tile([128, 512], F32, tag="mm", name=f"psyB{p}")
            if tk >= 5 and (tk - 5) % 2 == 0 and nextq < NPAIR:
                emit_attnv_pair(p, nextq, ex.pop(nextq), ps_y2)
                nextq += 1
            if kwork and p > 0 and tk >= 4:
                o2, w2, ci, t = kwork.pop(0)
                emit_k_chunk_mm(kch[p + 1], p + 1, o2, w2, ci, t)
        while kwork:
            o2, w2, ci, t = kwork.pop(0)
            emit_k_chunk_mm(kch[p + 1], p + 1, o2, w2, ci, t)
        while nextq < NPAIR:
            emit_attnv_pair(p, nextq, ex.pop(nextq), ps_y2)
            nextq += 1
        pend_r.append(dict(p=p, ps_y2=ps_y2))
        if "k" in dbg:
            for (o2, w2) in KT:
                dk_ = P4.tile([128, 512], F32, tag="dbgt", name=f"dbk{p}_{o2}", bufs=1)
                nc.vector.tensor_copy(out=dk_[:, 0:w2], in_=kch[p][:, o2:o2 + w2])
                nc.sync.dma_start(out=dbg["k"][ts(p, 128), o2:o2 + w2],
                                  in_=dk_[:, 0:w2])
    pr_ = pend_r.pop()
    r65s = emit_recips(pr_["p"], pr_["ps_y2"])
    emit_deferred_epilogue(p=pr_["p"], ps_y2=pr_["ps_y2"], r65s=r65s)
    proj_pre = {}
    ps_us = ps_mm.tile([1, NQ], F32, tag="mm", name="ps_us")
    for c in range(C6):
        nc.tensor.matmul(ps_us, uproj_s[:, c:c + 1], y_sb[:, c, :],
                         start=(c == 0), stop=(c == C6 - 1))
    if "y" in dbg:
        for c in range(C6):
            dy_ = P4.tile([128, 512], F32, tag="dbgt", name=f"dby{c}", bufs=1)
            nc.vector.tensor_copy(out=dy_, in_=y_sb[:, c, :])
            nc.sync.dma_start(out=dbg["y"][ts(c, 128), :], in_=dy_)

    # ========== proj (fp8 DR, PE-centered) -> d2 = x2 - mean2 ==========
    d2_sb = P1.tile([128, C6, NQ], F32, tag="x2")
    # LN2 mean rows (need only u.y + sumx0)
    srow = P6.tile([1, NQ], F32, tag="rowf", bufs=4, name="srow2")
    nc.vector.scalar_tensor_tensor(out=srow, in0=ps_us, scalar=float(sbp),
                                   in1=sumx0, op0=OP.add, op1=OP.add)
    mrow2n = P6.tile([1, NQ], BF16, tag="row", bufs=6, name="mrow2n")
    nc.vector.tensor_scalar(out=mrow2n, in0=srow, scalar1=-WS / D,
                            scalar2=None, op0=OP.mult)
    m2row = P6.tile([1, NQ], BF16, tag="row", bufs=6, name="m2row")
    nc.vector.tensor_scalar(out=m2row, in0=srow, scalar1=1.0 / D,
                            scalar2=None, op0=OP.mult)

    sq2 = P4.tile([128, C6, NQ], FP8, tag="sq", name="sq2", bufs=2)
    for co in range(C6):
        if co in proj_pre:
            ps = proj_pre[co]
            nc.tensor.matmul(ps, wproj8[:, 4:6, ts(co, 128)],
                             y_sb[:, 4:6, :],
                             start=False, stop=False, perf_mode=DR)
        else:
            ps = ps_mm.tile([128, NQ], F32, tag="mm")
            for ci in range(0, C6, 2):
                nc.tensor.matmul(ps, wproj8[:, ci:ci + 2, ts(co, 128)],
                                 y_sb[:, ci:ci + 2, :],
                                 start=(ci == 0), stop=False, perf_mode=DR)
        # center in-psum: ps += 1 (x) (-WS m2)
        nc.tensor.matmul(ps, onesb[0:1, :], mrow2n, start=False, stop=True)
        nc.vector.scalar_tensor_tensor(out=d2_sb[:, co, :], in0=ps,
                                       scalar=1.0 / WS,
                                       in1=xb_sb[:, co, :],
                                       op0=OP.mult, op1=OP.add)
        nc.scalar.activation(sq2[:, co, :], d2_sb[:, co, :], AF.Square)

    if "x2" in dbg:
        for c in range(C6):
            dx2_ = P4.tile([128, 512], F32, tag="dbgt", name=f"dbx2{c}", bufs=1)
            nc.vector.tensor_copy(out=dx2_, in_=d2_sb[:, c, :])
            nc.sync.dma_start(out=dbg["x2"][ts(c, 128), :], in_=dx2_)

    # LN2 var + rstd (var = E[d2^2] exactly -- d2 already centered)
    ps_sq2 = ps_mm.tile([1, NQ], F32, tag="mm", name="ps_sq2")
    for ci in range(0, C6, 2):
        nc.tensor.matmul(ps_sq2, ones8, sq2[:, ci:ci + 2, :],
                         start=(ci == 0), stop=(ci == C6 - 2), perf_mode=DR)
    vrow2 = P6.tile([1, NQ], F32, tag="rowf", bufs=4, name="vrow2")
    nc.vector.tensor_scalar(out=vrow2, in0=ps_sq2, scalar1=1.0 / D,
                            scalar2=None, op0=OP.mult)
    srt2 = P6.tile([1, NQ], F32, tag="rowf", bufs=4, name="srt2")
    nc.scalar.activation(srt2, vrow2, AF.Sqrt, bias=eps1)
    # warm the gelu table while PE runs proj/fc1 lead-in
    warmg = P6.tile([1, 8], F32, tag="rowf", bufs=4, name="warmg")
    nc.scalar.activation(warmg, srt2[0:1, 0:8], AF.Gelu)
    rf2 = P6.tile([1, NQ], F32, tag="rowf", bufs=4, name="rf2")
    nc.vector.reciprocal_approx_fast(out=rf2, in_=srt2)
    rrow2 = P6.tile([1, NQ], BF16, tag="row", bufs=6, name="rrow2")
    nc.vector.tensor_copy(out=rrow2, in_=rf2)
    prb2 = ps_mm.tile([128, NQ], F32, tag="mm", name="prb2")
    nc.tensor.matmul(prb2, onesb[0:1, :], rrow2, start=True, stop=True)
    xn2 = P2.tile([128, C6, NQ], FP8, tag="xn2", bufs=1)
    for c in range(C6):
        nc.vector.tensor_tensor(xn2[:, c, :], d2_sb[:, c, :], prb2,
                                op=OP.mult)

    if "xn2" in dbg:
        for c in range(C6):
            dn_ = P4.tile([128, 512], F32, tag="dbgt", name=f"dbn{c}", bufs=1)
            nc.vector.tensor_copy(out=dn_, in_=xn2[:, c, :])
            nc.sync.dma_start(out=dbg["xn2"][ts(c, 128), :], in_=dn_)

    # ========== MLP: fc1 fp8 DR -> gelu; fc2 bf16 ==========
    h_sb = P2.tile([128, HO24, NQ], BF16, tag="h", bufs=1)
    wfc2b = None
    for ho in range(HO24):
        if ho == 12:
            wfc2b = P2.tile([128, 12, D], BF16, tag="wbig", bufs=2)
            nc.sync.dma_start(
                out=wfc2b,
                in_=wfc2_d.rearrange("(a p) o -> p a o", p=128)[:, 12:24, :])
        ps = ps_mm.tile([128, NQ], F32, tag="mm")
        for ci in range(0, C6, 2):
            nc.tensor.matmul(ps, wfc1_sb[:, ci:ci + 2, ts(ho, 128)],
                             xn2[:, ci:ci + 2, :],
                             start=(ci == 0), stop=(ci == C6 - 2), perf_mode=DR)
        nc.scalar.activation(h_sb[:, ho, :], ps, AF.Gelu,
                             bias=bfc1_s[:, ho:ho + 1], scale=1.0 / WS)
    for co in range(C6):
        ps = ps_mm.tile([128, NQ], F32, tag="mm")
        for ho in range(HO24):
            wsrc = wfc2a if ho < 12 else wfc2b
            nc.tensor.matmul(ps, wsrc[:, ho % 12, ts(co, 128)], h_sb[:, ho, :],
                             start=(ho == 0), stop=False)
        # de-center the residual: ps += 1 (x) m2 (out = d2 + m2 + mlp)
        nc.tensor.matmul(ps, onesb[0:1, :], m2row, start=False, stop=True)
        o = P2.tile([128, NQ], F32, tag="ot")
        nc.vector.scalar_tensor_tensor(out=o, in0=ps,
                                       scalar=bfc2_s[:, co:co + 1],
                                       in1=d2_sb[:, co, :],
                                       op0=OP.add, op1=OP.add)
        nc.sync.dma_start(out=out_d[ts(co, 128), :], in_=o)

    for cm in (ps_sc_cm, ps_mm_cm, P6_cm, P4_cm, P2_cm, P1_cm):
        cm.__exit__(None, None, None)


def _host_prep(x, mask, ln1_g, ln1_b, qkv_w, proj_w, proj_b, ln2_g, ln2_b,
               fc1_w, fc1_b, fc2_w, fc2_b):
    bf = ml_dtypes.bfloat16
    fp8 = ml_dtypes.float8_e4m3
    f32 = np.float32
    x = np.asarray(x, f32)
    mask = np.asarray(mask)
    qkv_w = np.asarray(qkv_w, f32)
    proj_w = np.asarray(proj_w, f32)
    fc1_w = np.asarray(fc1_w, f32)
    fc2_w = np.asarray(fc2_w, f32)
    ln1_g = np.asarray(ln1_g, f32); ln1_b = np.asarray(ln1_b, f32)
    ln2_g = np.asarray(ln2_g, f32); ln2_b = np.asarray(ln2_b, f32)
    proj_b = np.asarray(proj_b, f32)
    fc1_b = np.asarray(fc1_b, f32); fc2_b = np.asarray(fc2_b, f32)

    # the raw-G scores fold (rstd on the exp scale) needs zero qkv bias,
    # which holds iff ln1_b == 0 (the reference qkv matmul has no bias)
    assert np.all(ln1_b == 0.0), "kernel assumes ln1_b == 0"

    wqkv_f = qkv_w * ln1_g[None, :]
    wqkv_f[0:D] *= SCALE
    bproj_f = proj_b
    cs = wqkv_f.sum(axis=1)          # [2304] per-output-channel colsums
    wfc1_f = fc1_w * ln2_g[None, :]
    bfc1_f = fc1_w @ ln2_b + fc1_b

    shared = {
        "wqkv": np.ascontiguousarray(wqkv_f.T * WS).astype(fp8),
        "wproj": np.ascontiguousarray(proj_w.T * WS).astype(fp8),
        "wfc1": np.ascontiguousarray(wfc1_f.T * WS).astype(fp8),
        "wfc2": np.ascontiguousarray(fc2_w.T).astype(bf),
        "uproj": np.ascontiguousarray(
            proj_w.sum(axis=0).reshape(6, 128).T).astype(bf),
        "csrows": np.ascontiguousarray((-WS * cs)[None, :]).astype(bf),
    }
    sbp = float(bproj_f.sum())

    # compacted unmasked keys per batch
    kidx = [np.where(mask[b] != 1)[0] for b in range(B)]
    nk_max = max(len(i) for i in kidx)
    NK = max(((nk_max + 255) // 256) * 256, 512)

    # packed f32 constants: m01 (per-core) | csq | csk | bfc1 | bfc2
    constf_tail = np.concatenate([
        (-WS * cs[0:D]).reshape(6, 128).T,
        (-WS * cs[D:2 * D]).reshape(6, 128).T,
        bfc1_f.reshape(24, 128).T,
        fc2_b.reshape(6, 128).T,
    ], axis=1).astype(f32)

    in_maps = []
    for core in range(NC):
        b, s = divmod(core, NSH)
        xq = x[b][s * NQ:(s + 1) * NQ]       # [NQ, D]
        nk = len(kidx[b])
        xk = np.zeros((NK, D), f32)
        xk[:nk] = x[b][kidx[b]]
        m01k = np.zeros((NK,), f32)
        m01k[:nk] = 1.0
        im = dict(shared)
        im["xq8"] = np.ascontiguousarray(xq.T).astype(fp8)
        im["xk8"] = np.ascontiguousarray(xk.T).astype(fp8)
        im["xb"] = np.ascontiguousarray(xq.T + bproj_f[:, None]).astype(bf)
        im["constf"] = np.ascontiguousarray(np.concatenate(
            [m01k.reshape(NK // 128, 128).T.astype(f32), constf_tail],
            axis=1))
        in_maps.append(im)
    return in_maps, sbp, NK


def kernel(**inputs):
    in_maps, sbp, NK = _host_prep(**inputs)
    if _cached.get("key") != (sbp, NK):
        _cached["nc"] = _build_nc(sbp, NK)
        _cached["key"] = (sbp, NK)
    res = run_bass_kernel_spmd(_cached["nc"], in_maps, core_ids=list(range(NC)))
    out = np.empty((B, N, D), np.float32)
    for core in range(NC):
        b, s = divmod(core, NSH)
        out[b, s * NQ:(s + 1) * NQ, :] = res.results[core]["out"].T
    return out


# revision 33
# speedup vs baseline: 1.0301x; 1.0065x over previous
"""Trainium2 Bass kernel for a dense transformer block (pre-LN, MHA + GELU MLP).

Problem shapes (hardcoded): x [2, 2048, 768] f32, mask [2, 2048] int32,
12 heads x 64 dims, hidden 3072.

Sharding: 8 cores = (batch b in {0,1}) x (token shard s in {0..3}).
Each core handles its 512-query shard of Q / attention rows / MLP / output.

Key compaction: the key-padding mask kills ~half the keys, so the host
gathers only the unmasked keys' x columns into xk [D, NK] (NK = count
rounded up to 256, zero-padded). Pad keys are neutralized by zeroing their
V rows and ones-column entries.

LN1 is never applied to activations ("raw-G" trick): Q/K/V matmuls run on
raw fp8 x directly (so the PE starts as soon as DMA lands, with no
LN-apply serialization), and the normalization is folded into the
epilogues via per-output-channel weight column sums:
    q[o,t] = r[t] * (G[o,t] - m[t]*colsum_w[o])        (valid: ln1_b == 0)
The query-side rstd folds into the Q epilogue; the key-side rstd folds
into the exp's per-partition ACT scale (keys sit on PSUM partitions for
scores), and the V-side mean/rstd fold into the V epilogue using
transposed per-key mean/rstd columns (tiny PE transpose matmuls).

On-chip layout is feature-major: activations are [features, tokens],
every matmul contracts over the partition dim. Per-token LN stats come
from ones-vector DoubleRow matmuls on raw fp8 x; the fp8 squares are
split across DVE/ACT/GpSimd so no one engine serializes the startup.
Softmax denominators come free from a ones column appended to V (M=65
matmul). Max-subtraction is skipped: |scores| <= ~4 by construction.
LN2 mean is folded into the proj PSUM as a K=1 rank-1 accumulate (and
un-folded in the fc2 PSUM), so the residual stream lives centered (d2)
and the LN2 apply is one multiply.

fp8 DoubleRow everywhere except fc2 (kept bf16: quantizing the 3072-wide
contraction would eat too much of the 2e-2 error budget). DMAs are one
issue per tensor (descriptor issue costs ~600ns on the Sync engine) in
consumer-priority order. Perf note for this box: the PE runs at 1.2 GHz
through dependency-mixed regions (only long uniform MM streams reach
2.4 GHz), so attention-path epilogues belong on DVE, not on the PE.
"""

import numpy as np
import ml_dtypes

import concourse.bass as bass
import concourse.tile as tile
import concourse.mybir as mybir
from concourse import bacc
from concourse.bass import ts
from concourse.bass_utils import run_bass_kernel_spmd
from concourse.alu_op_type import AluOpType

BF16 = mybir.dt.bfloat16
F32 = mybir.dt.float32
FP8 = mybir.dt.float8e4
DR = mybir.MatmulPerfMode.DoubleRow
WS = 32.0   # fp8 weight scale (dodges e4m3 subnormals)

B = 2
N = 2048
D = 768
H = 12
HD = 64
HID = 3072
EPS = 1e-5
SCALE = HD ** -0.5
NQ = 512          # queries per core
NSH = N // NQ     # token shards per batch
NC = B * NSH      # 8 cores
C6 = D // 128     # feature chunks
HO24 = HID // 128

AF = mybir.ActivationFunctionType
OP = AluOpType

_cached = {}
_rid = [0]


def _rid_next():
    _rid[0] += 1
    return _rid[0]


def _build_nc(sbp, NK):
    nc = bacc.Bacc("TRN2", target_bir_lowering=False, debug=False,
                   enable_asserts=False, num_devices=NC)

    xq8 = nc.dram_tensor("xq8", [D, NQ], FP8, kind="ExternalInput").ap()
    xk8 = nc.dram_tensor("xk8", [D, NK], FP8, kind="ExternalInput").ap()
    xb = nc.dram_tensor("xb", [D, NQ], BF16, kind="ExternalInput").ap()
    wqkv = nc.dram_tensor("wqkv", [D, 3 * D], FP8, kind="ExternalInput").ap()
    wproj = nc.dram_tensor("wproj", [D, D], FP8, kind="ExternalInput").ap()
    wfc1 = nc.dram_tensor("wfc1", [D, HID], FP8, kind="ExternalInput").ap()
    wfc2 = nc.dram_tensor("wfc2", [HID, D], BF16, kind="ExternalInput").ap()
    uproj = nc.dram_tensor("uproj", [128, 6], BF16, kind="ExternalInput").ap()
    # negated WS-scaled weight column sums as rows: csq | csk | csv
    csrows = nc.dram_tensor("csrows", [1, 3 * D], BF16,
                            kind="ExternalInput").ap()
    KC = NK // 128
    # packed f32 constants: m01 | csq | csk | bfc1 | bfc2
    constf = nc.dram_tensor("constf", [128, KC + 42], F32,
                            kind="ExternalInput").ap()
    out_d = nc.dram_tensor("out", [D, NQ], F32, kind="ExternalOutput").ap()
    import os
    dbg = {}
    if os.environ.get("KDBG"):
        dbg["y"] = nc.dram_tensor("dbg_y", [D, NQ], F32, kind="ExternalOutput").ap()
        dbg["x2"] = nc.dram_tensor("dbg_x2", [D, NQ], F32, kind="ExternalOutput").ap()
        dbg["q"] = nc.dram_tensor("dbg_q", [D, NQ], F32, kind="ExternalOutput").ap()
        dbg["k"] = nc.dram_tensor("dbg_k", [D, NK], F32, kind="ExternalOutput").ap()
        dbg["xn2"] = nc.dram_tensor("dbg_xn2", [D, NQ], F32, kind="ExternalOutput").ap()

    with tile.TileContext(nc) as tc:
        _body(nc, tc, sbp, NK, xq8, xk8, xb, wqkv, wproj, wfc1, wfc2,
              uproj, csrows, constf, out_d, dbg)
    nc.compile()
    return nc


def _body(nc, tc, sbp, NK, xq8_d, xk8_d, xb_d, wqkv_d, wproj_d, wfc1_d,
          wfc2_d, uproj_d, csrows_d, constf_d, out_d, dbg=None):
    dbg = dbg or {}
    KC = NK // 128
    NPAIR = KC // 2
    KT = []
    off = 0
    while off < NK:
        w = min(512, NK - off)
        KT.append((off, w))
        off += w
    NT = len(KT)

    P1_cm = tc.tile_pool(name="p1", bufs=1); P1 = P1_cm.__enter__()
    P2_cm = tc.tile_pool(name="p2", bufs=2); P2 = P2_cm.__enter__()
    P4_cm = tc.tile_pool(name="p4", bufs=4); P4 = P4_cm.__enter__()
    P6_cm = tc.tile_pool(name="p6", bufs=6); P6 = P6_cm.__enter__()
    ps_mm_cm = tc.tile_pool(name="ps_mm", bufs=4, space="PSUM")
    ps_mm = ps_mm_cm.__enter__()
    ps_sc_cm = tc.tile_pool(name="ps_sc", bufs=2, space="PSUM")
    ps_sc = ps_sc_cm.__enter__()

    # ---- DMAs: one issue per tensor (issue cost ~600ns each on Sync),
    # ordered by when the consumer needs the data ----
    xq8_r = xq8_d.rearrange("(a p) t -> p a t", p=128)
    xk8_r = xk8_d.rearrange("(a p) t -> p a t", p=128)
    wqkv_r = wqkv_d.rearrange("(a p) o -> p a o", p=128)
    x8 = P1.tile([128, C6, NQ], FP8, tag="x8")
    nc.sync.dma_start(out=x8[:, 0:2, :], in_=xq8_r[:, 0:2, :])
    nc.sync.dma_start(out=x8[:, 2:C6, :], in_=xq8_r[:, 2:C6, :])
    w_sb = P2.tile([128, C6, 3 * D], FP8, tag="wbig", bufs=2)
    nc.sync.dma_start(out=w_sb[:, :, 0:D], in_=wqkv_r[:, :, 0:D])
    xk8 = P1.tile([128, C6, NK], FP8, tag="xk8")
    for (o2, w2) in KT:
        nc.sync.dma_start(out=xk8[:, :, o2:o2 + w2], in_=xk8_r[:, :, o2:o2 + w2])
    # packed f32 constants: m01 | csq | csk | bfc1 | bfc2
    constf = P1.tile([128, KC + 42], F32, tag="constf")
    nc.sync.dma_start(out=constf, in_=constf_d)
    m01 = constf[:, 0:KC]
    csq_s = constf[:, KC:KC + 6]
    csk_s = constf[:, KC + 6:KC + 12]
    bfc1_s = constf[:, KC + 12:KC + 36]
    bfc2_s = constf[:, KC + 36:KC + 42]
    nc.sync.dma_start(out=w_sb[:, :, D:2 * D], in_=wqkv_r[:, :, D:2 * D])
    csr = P1.tile([1, 3 * D], BF16, tag="csr")
    nc.sync.dma_start(out=csr, in_=csrows_d)
    csq_row = csr[:, 0:D]
    csk_row = csr[:, D:2 * D]
    csv_row = csr[:, 2 * D:3 * D]
    nc.sync.dma_start(out=w_sb[:, :, 2 * D:3 * D], in_=wqkv_r[:, :, 2 * D:3 * D])
    uproj_s = P1.tile([128, 6], BF16, tag="uproj")
    nc.sync.dma_start(out=uproj_s, in_=uproj_d)

    # ---- on-chip constants ----
    onesb = P1.tile([128, 128], BF16, tag="onesb")
    nc.vector.memset(onesb, 1.0)
    ones8_t = P1.tile([128, 2, 16], FP8, tag="ones8")
    nc.vector.memset(ones8_t, 1.0)
    ones8 = ones8_t[:, :, 0:1]
    eps1 = P1.tile([1, 1], F32, tag="eps1")
    nc.vector.memset(eps1, EPS)
    sumx0 = P1.tile([1, NQ], BF16, tag="sumx0")

    # ---- later-phase DMAs (behind the attention-critical ones) ----
    xb_sb = P1.tile([128, C6, NQ], BF16, tag="xb")
    nc.sync.dma_start(out=xb_sb, in_=xb_d.rearrange("(a p) t -> p a t", p=128))
    wproj8 = P1.tile([128, C6, D], FP8, tag="wpj")
    nc.sync.dma_start(out=wproj8, in_=wproj_d.rearrange("(a p) o -> p a o", p=128))
    wfc1_sb = P1.tile([128, C6, HID], FP8, tag="wfc1")
    nc.sync.dma_start(out=wfc1_sb, in_=wfc1_d.rearrange("(a p) o -> p a o", p=128))
    wfc2a = P2.tile([128, 12, D], BF16, tag="wbig", bufs=2)
    nc.sync.dma_start(out=wfc2a,
                      in_=wfc2_d.rearrange("(a p) o -> p a o", p=128)[:, 0:12, :])

    # ========== LN stats on raw fp8 x (sum/sumsq via DR matmuls) ==========
    def emit_stats(src, t_off, w, nm, sqeng="dve"):
        i = _rid_next()
        ps_sum = ps_mm.tile([1, w], F32, tag="mm", name=f"pssum{i}")
        ps_sq = ps_mm.tile([1, w], F32, tag="mm", name=f"pssq{i}")
        for ci in range(0, C6, 2):
            nc.tensor.matmul(ps_sum, ones8, src[:, ci:ci + 2, t_off:t_off + w],
                             start=(ci == 0), stop=(ci == C6 - 2), perf_mode=DR)
        sq = P4.tile([128, C6, w], FP8, tag="sq", name=f"sq{i}", bufs=2)
        for ci in range(0, C6, 2):
            if sqeng == "act":
                nc.scalar.activation(sq[:, ci:ci + 2, :],
                                     src[:, ci:ci + 2, t_off:t_off + w],
                                     AF.Square)
            elif sqeng == "gps":
                nc.gpsimd.tensor_tensor(sq[:, ci:ci + 2, :],
                                        src[:, ci:ci + 2, t_off:t_off + w],
                                        src[:, ci:ci + 2, t_off:t_off + w],
                                        op=OP.mult)
            else:
                nc.vector.tensor_tensor(sq[:, ci:ci + 2, :],
                                        src[:, ci:ci + 2, t_off:t_off + w],
                                        src[:, ci:ci + 2, t_off:t_off + w],
                                        op=OP.mult)
        for ci in range(0, C6, 2):
            nc.tensor.matmul(ps_sq, ones8, sq[:, ci:ci + 2, :],
                             start=(ci == 0), stop=(ci == C6 - 2), perf_mode=DR)
        mrow = P6.tile([1, w], BF16, tag="row", bufs=6, name=f"mrow{i}")
        nc.vector.tensor_scalar(out=mrow, in0=ps_sum, scalar1=1.0 / D,
                                scalar2=None, op0=OP.mult)
        m2 = P6.tile([1, w], F32, tag="rowf", bufs=4, name=f"m2_{i}")
        nc.vector.tensor_tensor(m2, mrow, mrow, op=OP.mult)
        vrow = P6.tile([1, w], F32, tag="rowf", bufs=4, name=f"vrow{i}")
        nc.vector.scalar_tensor_tensor(out=vrow, in0=ps_sq, scalar=1.0 / D,
                                       in1=m2, op0=OP.mult, op1=OP.subtract)
        srt = P6.tile([1, w], F32, tag="rowf", bufs=4, name=f"srt{i}")
        nc.scalar.activation(srt, vrow, AF.Sqrt, bias=eps1)
        rf = P6.tile([1, w], F32, tag="rowf", bufs=4, name=f"rf{i}")
        nc.vector.reciprocal_approx_fast(out=rf, in_=srt)
        rrow = P6.tile([1, w], BF16, tag="row", bufs=6, name=f"rrow{i}")
        nc.vector.tensor_copy(out=rrow, in_=rf)
        return ps_sum, mrow, rrow

    # ---- broadcast tiles (PE K=1 matmul + copy) ----
    def emit_bcast(row, w, nm):
        pmb = ps_mm.tile([128, w], F32, tag="mm", name=f"pmb_{nm}")
        nc.tensor.matmul(pmb, onesb[0:1, :], row, start=True, stop=True)
        t = P4.tile([128, w], BF16, tag="bcs", name=f"bc_{nm}", bufs=4)
        nc.vector.tensor_copy(out=t, in_=pmb)
        return t

    # ---- per-key mean/rstd columns via tiny PE transposes ----
    mkc = P1.tile([128, KC], F32, tag="mkc")
    expsc = P1.tile([128, KC], F32, tag="expsc")
    vscale = P1.tile([128, KC], F32, tag="vscale")
    avcol = P1.tile([128, KC], F32, tag="avcol")

    def emit_kcols(t):
        o2, w2 = KT[t]
        nch = w2 // 128
        c0 = o2 // 128
        i = _rid_next()
        pm = ps_mm.tile([128, nch], F32, tag="mm", name=f"pstm{i}")
        pr = ps_mm.tile([128, nch], F32, tag="mm", name=f"pstr{i}")
        for k in range(nch):
            nc.tensor.matmul(pm[:, k:k + 1],
                             kstats[t][1][0:1, k * 128:(k + 1) * 128],
                             onesb[0:1, 0:1], start=True, stop=True)
            nc.tensor.matmul(pr[:, k:k + 1],
                             kstats[t][2][0:1, k * 128:(k + 1) * 128],
                             onesb[0:1, 0:1], start=True, stop=True)
        nc.vector.tensor_copy(out=mkc[:, c0:c0 + nch], in_=pm)
        nc.vector.tensor_scalar(out=expsc[:, c0:c0 + nch], in0=pr,
                                scalar1=1.0 / WS, scalar2=None, op0=OP.mult)
        nc.vector.tensor_tensor(vscale[:, c0:c0 + nch], expsc[:, c0:c0 + nch],
                                m01[:, c0:c0 + nch], op=OP.mult)
        nc.vector.tensor_tensor(avcol[:, c0:c0 + nch], mkc[:, c0:c0 + nch],
                                vscale[:, c0:c0 + nch], op=OP.mult)

    # stats: queries, then key tile 0 (tile 1 deferred until after the
    # first-scores dependencies so exp can start ASAP)
    ps_sum_q, mrow_q, rrow_q = emit_stats(x8, 0, NQ, "q")
    nc.vector.tensor_copy(out=sumx0, in_=ps_sum_q)
    # V+denominator store; ones/mask column seeded here: the ~2us strided
    # fp8 scatter runs in a DVE-idle window, off the row-chain path
    vsb = P1.tile([128, KC, 16 * ((H * (HD + 1) + 15) // 16)], FP8, tag="vsb")
    m01r = bass.AP(tensor=m01.tensor, offset=m01.offset,
                   ap=[list(m01.ap[0]), list(m01.ap[1]), [0, H], [0, 1]])
    vsb_h = vsb[:, :, 0:H * (HD + 1)].rearrange("p k (h e) -> p k h e",
                                                e=HD + 1)
    nc.vector.tensor_copy(out=vsb_h[:, :, :, HD:HD + 1], in_=m01r)
    kstats = [None] * NT
    mb_k = [None] * NT
    for t in range(NT):
        kstats[t] = emit_stats(xk8, KT[t][0], KT[t][1], f"k{t}", sqeng="act")
        mb_k[t] = emit_bcast(kstats[t][1], KT[t][1], f"mk{t}")
        emit_kcols(t)
    mb_q = emit_bcast(mrow_q, NQ, "mq")
    rbrow = P6.tile([1, NQ], BF16, tag="row", bufs=6, name="rbrow")
    nc.vector.tensor_scalar(out=rbrow, in0=rrow_q, scalar1=1.0 / WS,
                            scalar2=None, op0=OP.mult)
    rb_q = emit_bcast(rbrow, NQ, "rq")

    # ========== Q (raw-G + folded epilogue) ==========
    qT = P2.tile([128, C6, NQ], BF16, tag="qT", bufs=1)
    for co in range(C6):
        ps = ps_mm.tile([128, NQ], F32, tag="mm")
        for ci in range(0, C6, 2):
            nc.tensor.matmul(ps, w_sb[:, ci:ci + 2, ts(co, 128)],
                             x8[:, ci:ci + 2, :],
                             start=(ci == 0), stop=(ci == C6 - 2), perf_mode=DR)
        t1 = P4.tile([128, NQ], BF16, tag="tmp", name=f"qt1_{co}")
        nc.vector.scalar_tensor_tensor(out=t1, in0=mb_q,
                                       scalar=csq_s[:, co:co + 1], in1=ps,
                                       op0=OP.mult, op1=OP.add)
        nc.vector.tensor_tensor(qT[:, co, :], t1, rb_q, op=OP.mult)

    if "q" in dbg:
        for c in range(C6):
            dq_ = P4.tile([128, 512], F32, tag="dbgt", name=f"dbq{c}", bufs=1)
            nc.vector.tensor_copy(out=dq_, in_=qT[:, c, :])
            nc.sync.dma_start(out=dbg["q"][ts(c, 128), :], in_=dq_)

    # ========== attention pipeline ==========

    kch_state = [None]

    def emit_k_chunk_mm(kch_p, p, o2, w2, ci, t):
        if ci == 0:
            kst = ps_mm.tile([128, w2], F32, tag="mm", name=f"kst{_rid_next()}")
            kch_state[0] = kst
        nc.tensor.matmul(kch_state[0], w_sb[:, ci:ci + 2, ts(6 + p, 128)],
                         xk8[:, ci:ci + 2, o2:o2 + w2],
                         start=(ci == 0), stop=(ci == C6 - 2), perf_mode=DR)
        if ci == C6 - 2:
            nc.vector.scalar_tensor_tensor(out=kch_p[:, o2:o2 + w2],
                                           in0=mb_k[t],
                                           scalar=csk_s[:, p:p + 1],
                                           in1=kch_state[0],
                                           op0=OP.mult, op1=OP.add)
            kch_state[0] = None

    def emit_v_chunk(tk):
        i = _rid_next()
        for half in range(2):
            psv = ps_mm.tile([128, 512], F32, tag="mm", name=f"psv{tk}_{half}")
            for ci in range(0, C6, 2):
                nc.tensor.matmul(psv[:, 0:384],
                                 xk8[:, ci:ci + 2, ts(tk, 128)],
                                 w_sb[:, ci:ci + 2, 12 * 128 + half * 384:
                                      12 * 128 + (half + 1) * 384],
                                 start=(ci == 0), stop=(ci == C6 - 2),
                                 perf_mode=DR)
            t1 = P4.tile([128, 384], BF16, tag="vt", name=f"vt{i}_{half}",
                         bufs=4)
            nc.vector.tensor_scalar(out=t1, in0=psv[:, 0:384],
                                    scalar1=vscale[:, tk:tk + 1],
                                    scalar2=None, op0=OP.mult)
            vout = vsb[:, tk, half * 390:half * 390 + 390].rearrange(
                "p (h e) -> p h e", e=HD + 1)[:, :, 0:HD]
            vin = t1.rearrange("p (h d) -> p h d", h=6)
            csvv = csv_t[:, half, :].rearrange("p (h d) -> p h d", h=6)
            nc.vector.scalar_tensor_tensor(out=vout, in0=csvv,
                                           scalar=avcol[:, tk:tk + 1],
                                           in1=vin, op0=OP.mult, op1=OP.add)

    def emit_attnv_pair(p, q, ex2t, ps_y2):
        for j in range(2):
            h = 2 * p + j
            nc.tensor.matmul(ps_y2[j][0:HD + 1, :],
                             vsb[:, 2 * q:2 * q + 2, h * 65:h * 65 + 65],
                             ex2t[:, :, j, :],
                             start=(q == 0), stop=(q == NPAIR - 1),
                             perf_mode=DR)

    def emit_recips(p, ps_y2, pool=None, ptag="mm"):
        pool = pool or ps_mm
        r65s = []
        for j in range(2):
            sr = P4.tile([128, 512], BF16, tag="tf", name=f"sr{p}_{j}")
            nc.vector.tensor_copy(out=sr[HD:HD + 1, :],
                                  in_=ps_y2[j][HD:HD + 1, :])
            psb = pool.tile([64, 512], F32, tag=ptag, name=f"psb{p}_{j}")
            nc.tensor.matmul(psb, onesb[HD:HD + 1, 0:HD], sr[HD:HD + 1, :],
                             start=True, stop=True)
            rbf = P4.tile([128, 512], F32, tag="tf", name=f"rbf{p}_{j}")
            nc.vector.reciprocal_approx_fast(out=rbf[0:HD, :], in_=psb[0:HD, :])
            r65s.append(rbf)
        return r65s

    def emit_deferred_epilogue(p, ps_y2, r65s):
        for j in range(2):
            ps_y = ps_y2[j]
            if j == 0:
                nc.vector.tensor_tensor(y_sb[0:HD, p, :], ps_y[0:HD, :],
                                        r65s[j][0:HD, :], op=OP.mult)
            else:
                yt = P4.tile([128, 512], FP8, tag="yt", name=f"yt{p}")
                nc.vector.tensor_tensor(yt[0:HD, :], ps_y[0:HD, :],
                                        r65s[j][0:HD, :], op=OP.mult)
                nc.sync.dma_start(out=y_sb[HD:128, p, :], in_=yt[0:HD, :])

    y_sb = P1.tile([128, C6, NQ], FP8, tag="y")
    kch = {}
    pend = []
    pend_r = []

    # K(0) tile 0 first (unblocks scores/exp), then deferred key-tile-1
    # stats, then K(0) tile 1; V chunks go inside the p=0 loop
    kch[0] = P2.tile([128, NK], BF16, tag="kch", name="kch0")
    for t in range(NT):
        for ci in range(0, C6, 2):
            emit_k_chunk_mm(kch[0], 0, KT[t][0], KT[t][1], ci, t)
    # csv broadcast tile [128, 2, 384] (negated, x WS host-side)
    csv_t = P1.tile([128, 2, 384], BF16, tag="csvt")
    for half in range(2):
        pmcv = ps_mm.tile([128, 384], F32, tag="mm", name=f"pmcsv{half}")
        nc.tensor.matmul(pmcv, onesb[0:1, :],
                         csv_row[0:1, half * 384:(half + 1) * 384],
                         start=True, stop=True)
        nc.vector.tensor_copy(out=csv_t[:, half, :], in_=pmcv)
    # warm the exp ACT table (after the LN1 Sqrt set, before first exp)
    warme = P1.tile([1, 8], F32, tag="warme")
    nc.scalar.activation(warme, onesb[0:1, 0:8], AF.Exp)
    emit_v_chunk(0)
    emit_v_chunk(1)
    emit_v_chunk(2)
    emit_v_chunk(3)

    for p in range(C6):
        if p < C6 - 1:
            kch[p + 1] = P2.tile([128, NK], BF16, tag="kch", name=f"kch{p + 1}")
            kwork = [(o2, w2, ci, t) for t, (o2, w2) in enumerate(KT)
                     for ci in range(0, C6, 2)]
        else:
            kwork = []
        ex = {}
        ps_y2 = [None, None]
        nextq = 0
        for tk in range(KC):
            pss = ps_sc.tile([128, 2, 512], F32, tag="sc")
            for j in range(2):
                po = j * 64
                nc.tensor.matmul(pss[:, j, :],
                                 kch[p][po:po + 64, ts(tk, 128)],
                                 qT[po:po + 64, p, 0:NQ],
                                 start=True, stop=True)
            if tk % 2 == 0:
                ex[tk // 2] = P6.tile([128, 2, 2, 512], FP8, tag="exp",
                                      name=f"ex_{p}_{tk // 2}", bufs=4)
            nc.scalar.activation(ex[tk // 2][:, tk % 2, :, :], pss, AF.Exp,
                                 scale=expsc[:, tk:tk + 1])
            if p == 0 and 0 <= tk <= KC - 5:
                emit_v_chunk(tk + 4)
            # recips/epilogue of the PREVIOUS p are emitted here, behind this
            # p's first scores, so the PE never stalls on the DVE row chain
            # at the p-boundary (the stall tripped the clock-gate every p)
            if tk == 1 and pend_r:
                pr_ = pend_r.pop()
                r65s = emit_recips(pr_["p"], pr_["ps_y2"])
                pend.append(dict(p=pr_["p"], ps_y2=pr_["ps_y2"], r65s=r65s))
            if tk == 3 and pend:
                emit_deferred_epilogue(**pend.pop())
            if tk == 5:
                ps_y2[0] = ps_mm.tile([128, 512], F32, tag="mm", name=f"psyA{p}")
                ps_y2[1] = ps_mm.tile([128, 512], F32, tag="mm", name=f"psyB{p}")
            if tk >= 5 and (tk - 5) % 2 == 0 and nextq < NPAIR:
                emit_attnv_pair(p, nextq, ex.pop(nextq), ps_y2)
                nextq += 1
            if kwork and p > 0 and tk >= 4:
                o2, w2, ci, t = kwork.pop(0)
                emit_k_chunk_mm(kch[p + 1], p + 1, o2, w2, ci, t)
        while kwork:
            o2, w2, ci, t = kwork.pop(0)
            emit_k_chunk_mm(kch[p + 1], p + 1, o2, w2, ci, t)
        while nextq < NPAIR:
            emit_attnv_pair(p, nextq, ex.pop(nextq), ps_y2)
            nextq += 1
        pend_r.append(dict(p=p, ps_y2=ps_y2))
        if "k" in dbg:
            for (o2, w2) in KT:
                dk_ = P4.tile([128, 512], F32, tag="dbgt", name=f"dbk{p}_{o2}", bufs=1)
                nc.vector.tensor_copy(out=dk_[:, 0:w2], in_=kch[p][:, o2:o2 + w2])
                nc.sync.dma_start(out=dbg["k"][ts(p, 128), o2:o2 + w2],
                                  in_=dk_[:, 0:w2])
    pr_ = pend_r.pop()
    r65s = emit_recips(pr_["p"], pr_["ps_y2"])
    emit_deferred_epilogue(p=pr_["p"], ps_y2=pr_["ps_y2"], r65s=r65s)
    proj_pre = {}
    ps_us = ps_mm.tile([1, NQ], F32, tag="mm", name="ps_us")
    for c in range(C6):
        nc.tensor.matmul(ps_us, uproj_s[:, c:c + 1], y_sb[:, c, :],
                         start=(c == 0), stop=(c == C6 - 1))
    if "y" in dbg:
        for c in range(C6):
            dy_ = P4.tile([128, 512], F32, tag="dbgt", name=f"dby{c}", bufs=1)
            nc.vector.tensor_copy(out=dy_, in_=y_sb[:, c, :])
            nc.sync.dma_start(out=dbg["y"][ts(c, 128), :], in_=dy_)

    # ========== proj (fp8 DR, PE-centered) -> d2 = x2 - mean2 ==========
    d2_sb = P1.tile([128, C6, NQ], F32, tag="x2")
    # LN2 mean rows (need only u.y + sumx0)
    srow = P6.tile([1, NQ], F32, tag="rowf", bufs=4, name="srow2")
    nc.vector.scalar_tensor_tensor(out=srow, in0=ps_us, scalar=float(sbp),
                                   in1=sumx0, op0=OP.add, op1=OP.add)
    mrow2n = P6.tile([1, NQ], BF16, tag="row", bufs=6, name="mrow2n")
    nc.vector.tensor_scalar(out=mrow2n, in0=srow, scalar1=-WS / D,
                            scalar2=None, op0=OP.mult)
    m2row = P6.tile([1, NQ], BF16, tag="row", bufs=6, name="m2row")
    nc.vector.tensor_scalar(out=m2row, in0=srow, scalar1=1.0 / D,
                            scalar2=None, op0=OP.mult)

    sq2 = P4.tile([128, C6, NQ], FP8, tag="sq", name="sq2", bufs=2)
    for co in range(C6):
        if co in proj_pre:
            ps = proj_pre[co]
            nc.tensor.matmul(ps, wproj8[:, 4:6, ts(co, 128)],
                             y_sb[:, 4:6, :],
                             start=False, stop=False, perf_mode=DR)
        else:
            ps = ps_mm.tile([128, NQ], F32, tag="mm")
            for ci in range(0, C6, 2):
                nc.tensor.matmul(ps, wproj8[:, ci:ci + 2, ts(co, 128)],
                                 y_sb[:, ci:ci + 2, :],
                                 start=(ci == 0), stop=False, perf_mode=DR)
        # center in-psum: ps += 1 (x) (-WS m2)
        nc.tensor.matmul(ps, onesb[0:1, :], mrow2n, start=False, stop=True)
        nc.vector.scalar_tensor_tensor(out=d2_sb[:, co, :], in0=ps,
                                       scalar=1.0 / WS,
                                       in1=xb_sb[:, co, :],
                                       op0=OP.mult, op1=OP.add)
        nc.scalar.activation(sq2[:, co, :], d2_sb[:, co, :], AF.Square)

    if "x2" in dbg:
        for c in range(C6):
            dx2_ = P4.tile([128, 512], F32, tag="dbgt", name=f"dbx2{c}", bufs=1)
            nc.vector.tensor_copy(out=dx2_, in_=d2_sb[:, c, :])
            nc.sync.dma_start(out=dbg["x2"][ts(c, 128), :], in_=dx2_)

    # LN2 var + rstd (var = E[d2^2] exactly -- d2 already centered)
    ps_sq2 = ps_mm.tile([1, NQ], F32, tag="mm", name="ps_sq2")
    for ci in range(0, C6, 2):
        nc.tensor.matmul(ps_sq2, ones8, sq2[:, ci:ci + 2, :],
                         start=(ci == 0), stop=(ci == C6 - 2), perf_mode=DR)
    vrow2 = P6.tile([1, NQ], F32, tag="rowf", bufs=4, name="vrow2")
    nc.vector.tensor_scalar(out=vrow2, in0=ps_sq2, scalar1=1.0 / D,
                            scalar2=None, op0=OP.mult)
    srt2 = P6.tile([1, NQ], F32, tag="rowf", bufs=4, name="srt2")
    nc.scalar.activation(srt2, vrow2, AF.Sqrt, bias=eps1)
    # warm the gelu table while PE runs proj/fc1 lead-in
    warmg = P6.tile([1, 8], F32, tag="rowf", bufs=4, name="warmg")
    nc.scalar.activation(warmg, srt2[0:1, 0:8], AF.Gelu)
    rf2 = P6.tile([1, NQ], F32, tag="rowf", bufs=4, name="rf2")
    nc.vector.reciprocal_approx_fast(out=rf2, in_=srt2)
    rrow2 = P6.tile([1, NQ], BF16, tag="row", bufs=6, name="rrow2")
    nc.vector.tensor_copy(out=rrow2, in_=rf2)
    prb2 = ps_mm.tile([128, NQ], F32, tag="mm", name="prb2")
    nc.tensor.matmul(prb2, onesb[0:1, :], rrow2, start=True, stop=True)
    xn2 = P2.tile([128, C6, NQ], FP8, tag="xn2", bufs=1)
    for c in range(C6):
        nc.vector.tensor_tensor(xn2[:, c, :], d2_sb[:, c, :], prb2,
                                op=OP.mult)

    if "xn2" in dbg:
        for c in range(C6):
            dn_ = P4.tile([128, 512], F32, tag="dbgt", name=f"dbn{c}", bufs=1)
            nc.vector.tensor_copy(out=dn_, in_=xn2[:, c, :])
            nc.sync.dma_start(out=dbg["xn2"][ts(c, 128), :], in_=dn_)

    # ========== MLP: fc1 fp8 DR -> gelu; fc2 bf16 ==========
    h_sb = P2.tile([128, HO24, NQ], BF16, tag="h", bufs=1)
    wfc2b = None
    for ho in range(HO24):
        if ho == 12:
            wfc2b = P2.tile([128, 12, D], BF16, tag="wbig", bufs=2)
            nc.sync.dma_start(
                out=wfc2b,
                in_=wfc2_d.rearrange("(a p) o -> p a o", p=128)[:, 12:24, :])
        ps = ps_mm.tile([128, NQ], F32, tag="mm")
        for ci in range(0, C6, 2):
            nc.tensor.matmul(ps, wfc1_sb[:, ci:ci + 2, ts(ho, 128)],
                             xn2[:, ci:ci + 2, :],
                             start=(ci == 0), stop=(ci == C6 - 2), perf_mode=DR)
        nc.scalar.activation(h_sb[:, ho, :], ps, AF.Gelu,
                             bias=bfc1_s[:, ho:ho + 1], scale=1.0 / WS)
    for co in range(C6):
        ps = ps_mm.tile([128, NQ], F32, tag="mm")
        for ho in range(HO24):
            wsrc = wfc2a if ho < 12 else wfc2b
            nc.tensor.matmul(ps, wsrc[:, ho % 12, ts(co, 128)], h_sb[:, ho, :],
                             start=(ho == 0), stop=False)
        # de-center the residual: ps += 1 (x) m2 (out = d2 + m2 + mlp)
        nc.tensor.matmul(ps, onesb[0:1, :], m2row, start=False, stop=True)
        o = P2.tile([128, NQ], F32, tag="ot")
        nc.vector.scalar_tensor_tensor(out=o, in0=ps,
                                       scalar=bfc2_s[:, co:co + 1],
                                       in1=d2_sb[:, co, :],
                                       op0=OP.add, op1=OP.add)
        nc.sync.dma_start(out=out_d[ts(co, 128), :], in_=o)

    for cm in (ps_sc_cm, ps_mm_cm, P6_cm, P4_cm, P2_cm, P1_cm):
        cm.__exit__(None, None, None)


def _host_prep(x, mask, ln1_g, ln1_b, qkv_w, proj_w, proj_b, ln2_g, ln2_b,
               fc1_w, fc1_b, fc2_w, fc2_b):
    bf = ml_dtypes.bfloat16
    fp8 = ml_dtypes.float8_e4m3
    f32 = np.float32
    x = np.asarray(x, f32)
    mask = np.asarray(mask)
    qkv_w = np.asarray(qkv_w, f32)
    proj_w = np.asarray(proj_w, f32)
    fc1_w = np.asarray(fc1_w, f32)
    fc2_w = np.asarray(fc2_w, f32)
    ln1_g = np.asarray(ln1_g, f32); ln1_b = np.asarray(ln1_b, f32)
    ln2_g = np.asarray(ln2_g, f32); ln2_b = np.asarray(ln2_b, f32)
    proj_b = np.asarray(proj_b, f32)
    fc1_b = np.asarray(fc1_b, f32); fc2_b = np.asarray(fc2_b, f32)

    # the raw-G scores fold (rstd on the exp scale) needs zero qkv bias,
    # which holds iff ln1_b == 0 (the reference qkv matmul has no bias)
    assert np.all(ln1_b == 0.0), "kernel assumes ln1_b == 0"

    wqkv_f = qkv_w * ln1_g[None, :]
    wqkv_f[0:D] *= SCALE
    bproj_f = proj_b
    cs = wqkv_f.sum(axis=1)          # [2304] per-output-channel colsums
    wfc1_f = fc1_w * ln2_g[None, :]
    bfc1_f = fc1_w @ ln2_b + fc1_b

    shared = {
        "wqkv": np.ascontiguousarray(wqkv_f.T * WS).astype(fp8),
        "wproj": np.ascontiguousarray(proj_w.T * WS).astype(fp8),
        "wfc1": np.ascontiguousarray(wfc1_f.T * WS).astype(fp8),
        "wfc2": np.ascontiguousarray(fc2_w.T).astype(bf),
        "uproj": np.ascontiguousarray(
            proj_w.sum(axis=0).reshape(6, 128).T).astype(bf),
        "csrows": np.ascontiguousarray((-WS * cs)[None, :]).astype(bf),
    }
    sbp = float(bproj_f.sum())

    # compacted unmasked keys per batch
    kidx = [np.where(mask[b] != 1)[0] for b in range(B)]
    nk_max = max(len(i) for i in kidx)
    NK = max(((nk_max + 255) // 256) * 256, 512)

    # packed f32 constants: m01 (per-core) | csq | csk | bfc1 | bfc2
    constf_tail = np.concatenate([
        (-WS * cs[0:D]).reshape(6, 128).T,
        (-WS * cs[D:2 * D]).reshape(6, 128).T,
        bfc1_f.reshape(24, 128).T,
        fc2_b.reshape(6, 128).T,
    ], axis=1).astype(f32)

    in_maps = []
    for core in range(NC):
        b, s = divmod(core, NSH)
        xq = x[b][s * NQ:(s + 1) * NQ]       # [NQ, D]
        nk = len(kidx[b])
        xk = np.zeros((NK, D), f32)
        xk[:nk] = x[b][kidx[b]]
        m01k = np.zeros((NK,), f32)
        m01k[:nk] = 1.0
        im = dict(shared)
        im["xq8"] = np.ascontiguousarray(xq.T).astype(fp8)
        im["xk8"] = np.ascontiguousarray(xk.T).astype(fp8)
        im["xb"] = np.ascontiguousarray(xq.T + bproj_f[:, None]).astype(bf)
        im["constf"] = np.ascontiguousarray(np.concatenate(
            [m01k.reshape(NK // 128, 128).T.astype(f32), constf_tail],
            axis=1))
        in_maps.append(im)
    return in_maps, sbp, NK


def kernel(**inputs):
    in_maps, sbp, NK = _host_prep(**inputs)
    if _cached.get("key") != (sbp, NK):
        _cached["nc"] = _build_nc(sbp, NK)
        _cached["key"] = (sbp, NK)
    res = run_bass_kernel_spmd(_cached["nc"], in_maps, core_ids=list(range(NC)))
    out = np.empty((B, N, D), np.float32)
    for core in range(NC):
        b, s = divmod(core, NSH)
        out[b, s * NQ:(s + 1) * NQ, :] = res.results[core]["out"].T
    return out
